# revision 1
# baseline (speedup 1.0000x reference)
"""Multi-head attention (B=2,S=2048,D=1024,H=16,A=64) on 8 trn2 NeuronCores.

Sharding: core = 4*b + g  (b = batch, g = head-group of 4 heads).
Per core, everything flows in "transposed" (feature-on-partition) layout:
  qT,kT = matmuls of Wq/Wk vs xT;  v natural; S^T per head; softmax over keys
  via exp (no max-sub; scores ~ N(0,1)) with the denominator produced by a
  ones-column appended to v; normalized attT [C=256, S] per core.

Final output projection re-shard, two selectable modes:
  mode="a2a":  8-core mesh AllToAll (shard j = my attT columns for peer j's
               seq chunk; batch-duplicated since cross-batch pairs share
               nothing), then fc_out runs on BOTH received batch stacks and
               the host keeps the right one.
  mode="host": each core computes its partial fc_out over the full sequence
               of its batch; host sums the 4 partials per batch.
"""

import numpy as np

B, S, D, H, A = 2, 2048, 1024, 16, 64
GROUPS = 4              # head groups (cores per batch)
HPG = H // GROUPS       # heads per core = 4
C = HPG * A             # channels per core = 256
N_CORES = 8
SQ = S // GROUPS        # per-core output seq chunk = 512

MODE = "host"           # "a2a" or "host"
CDT = "bf16"            # matmul compute dtype: "f32r" or "bf16"


def build_nc(s=S, d=D, n_cores=N_CORES, mode=MODE, cdt=None):
    import concourse.bass as bass
    import concourse.mybir as mybir
    import concourse.tile as tile
    from concourse import bacc

    f32 = mybir.dt.float32
    cdt = cdt or CDT
    f32r = mybir.dt.bfloat16 if cdt == "bf16" else mybir.dt.float32r
    AF = mybir.ActivationFunctionType

    KD = d // 128        # d-tiles (contraction for projections)
    MC = C // 128        # c-tiles per core = 2
    NS = s // 128        # seq tiles
    QC = max(1, s // 512)  # q chunks for attention
    QW = s // QC         # q chunk width (<=512)
    sq = s // GROUPS     # out rows per core chunk
    KT_PER_ST = 2        # k-tiles packed per st/pt tile (exp batching)
    OW = min(512, d)     # output free-dim chunk
    NG = NS // KT_PER_ST # st/pt groups per (h,qc)

    def r(ap):
        return ap

    nc = bacc.Bacc(
        "TRN2", target_bir_lowering=False, debug=False,
        enable_asserts=True, num_devices=n_cores,
    )

    idt = f32r if cdt == "bf16" else f32   # host pre-converts in bf16 mode
    xT_d = nc.dram_tensor("xT", [d, s], idt, kind="ExternalInput").ap()
    wq_d = nc.dram_tensor("wq", [d, C], idt, kind="ExternalInput").ap()
    wk_d = nc.dram_tensor("wk", [d, C], idt, kind="ExternalInput").ap()
    wv_d = nc.dram_tensor("wv", [d, C], idt, kind="ExternalInput").ap()
    n_wo_rows = H * A if mode == "a2a" else C
    wo_d = nc.dram_tensor("wo", [n_wo_rows, d], idt, kind="ExternalInput").ap()
    hwdge = [nc.sync, nc.scalar, nc.gpsimd]

    def load(i, dst, src_ap):
        # bf16: plain HWDGE spread across engines; f32r: gpsimd casting DMA
        if cdt == "bf16":
            hwdge[i % 3].dma_start(dst, src_ap)
        else:
            nc.gpsimd.dma_start(dst, src_ap)
    bqs_d = nc.dram_tensor("bqs", [128, MC], f32, kind="ExternalInput").ap()
    bks_d = nc.dram_tensor("bks", [128, MC], f32, kind="ExternalInput").ap()
    bvb_d = nc.dram_tensor("bvb", [128, C], f32, kind="ExternalInput").ap()
    bob_d = nc.dram_tensor("bob", [128, d], f32, kind="ExternalInput").ap()
    if mode == "a2a":
        out0_d = nc.dram_tensor("out0", [sq, d], f32, kind="ExternalOutput").ap()
        out1_d = nc.dram_tensor("out1", [sq, d], f32, kind="ExternalOutput").ap()
    else:
        out_d = nc.dram_tensor("out", [s, d], f32, kind="ExternalOutput").ap()

    with tile.TileContext(nc) as tc:
        with tc.tile_pool(name="const", bufs=1) as cpool, \
             tc.tile_pool(name="qkv", bufs=1) as qpool, \
             tc.tile_pool(name="wop", bufs=1) as wopool, \
             tc.tile_pool(name="xTw", bufs=1) as xpool, \
             tc.tile_pool(name="ptp", bufs=6) as ptpool, \
             tc.tile_pool(name="sml", bufs=4) as spool, \
             tc.tile_pool(name="osb", bufs=3) as opool, \
             tc.tile_pool(name="fcp", bufs=1) as fcpool, \
             tc.tile_pool(name="ps1", bufs=2, space="PSUM") as pp1, \
             tc.tile_pool(name="pst", bufs=1, space="PSUM") as stp, \
             tc.tile_pool(name="pav", bufs=1, space="PSUM") as avp, \
             tc.tile_pool(name="dram", bufs=1, space="DRAM") as dpool:

            ones_f = cpool.tile([1, A], f32, name="ones_f")
            nc.vector.memset(ones_f[:], 1.0)
            ones_sb = cpool.tile([1, A], f32r, name="ones_sb")
            nc.scalar.copy(ones_sb[:], ones_f[:])
            bq_sb = cpool.tile([128, MC], f32, name="bq_sb")
            nc.sync.dma_start(bq_sb[:], bqs_d[:, :])
            bk_sb = cpool.tile([128, MC], f32, name="bk_sb")
            nc.sync.dma_start(bk_sb[:], bks_d[:, :])
            bvb_sb = cpool.tile([128, C], f32, name="bvb_sb")
            nc.sync.dma_start(bvb_sb[:], bvb_d[:, :])
            bob_sb = cpool.tile([128, d], f32, name="bob_sb")
            nc.sync.dma_start(bob_sb[:], bob_d[:, :])

            qT_sb = [qpool.tile([128, s], f32r, name=f"qT{mt}", tag=f"qT{mt}")
                     for mt in range(MC)]
            kT_sb = [qpool.tile([128, s], f32r, name=f"kT{mt}", tag=f"kT{mt}")
                     for mt in range(MC)]
            # v, padded per head with a ones column: [128, NS, HPG, A+1]
            v_sb = qpool.tile([128, NS, HPG, A + 1], f32r, name="v_sb", tag="v")
            vones_f = cpool.tile([128, NS * HPG], f32, name="vones_f")
            nc.vector.memset(vones_f[:], 1.0)
            nc.vector.tensor_copy(
                v_sb[:, :, :, A],
                vones_f[:].rearrange("p (t h) -> p t h", h=HPG))

            n_wo = n_wo_rows // 128
            wo_sb = [wopool.tile([128, d], f32r, name=f"wo{kt}", tag=f"wo{kt}")
                     for kt in range(n_wo)]
            attn_sb = [qpool.tile([128, s], f32r, name=f"attn{t}", tag=f"at{t}")
                       for t in range(MC)]

            # ---------------- input loads (interleaved issue) ----------------
            xT_sb = [xpool.tile([128, s], f32r, name=f"xT{kt}", tag=f"x{kt}")
                     for kt in range(KD)]
            w_sb = {}
            for wname in ("q", "k", "v"):
                w_sb[wname] = [
                    xpool.tile([128, C], f32r, name=f"w{wname}{kt}",
                               tag=f"w{wname}{kt}")
                    for kt in range(KD)
                ]
            wds = {"q": wq_d, "k": wk_d, "v": wv_d}
            li = 0
            for kt in range(KD):
                ks = slice(kt * 128, (kt + 1) * 128)
                load(li, xT_sb[kt][:], xT_d[ks, :]); li += 1
                for wname in ("q", "k", "v"):
                    load(li, w_sb[wname][kt][:], wds[wname][ks, :]); li += 1
            # wo only feeds the output projection — issue its loads last
            for kt in range(n_wo):
                load(li, wo_sb[kt][:], wo_d[kt * 128:(kt + 1) * 128, :])
                li += 1

            # ---------------- building blocks ----------------
            def project_qk(mt):
                # qT/kT[c, s] = sum_d W[d, c] * xT[d, s], bias via DVE
                for wname, dst, bias in (("q", qT_sb, bq_sb),
                                         ("k", kT_sb, bk_sb)):
                    for qc in range(QC):
                        qs = slice(qc * QW, (qc + 1) * QW)
                        ps = pp1.tile([128, QW], f32, name="ps_qk",
                                      tag="ps_qk")
                        for kt in range(KD):
                            nc.tensor.matmul(
                                ps[:],
                                lhsT=w_sb[wname][kt][:, mt * 128:(mt + 1) * 128],
                                rhs=xT_sb[kt][:, qs],
                                start=(kt == 0), stop=(kt == KD - 1),
                            )
                        nc.vector.tensor_scalar_add(
                            dst[mt][:, qs], ps[:], bias[:, mt:mt + 1])

            def project_v():
                # v[s_tile, c] = sum_d xT[d, s_tile] * Wv[d, c]
                for st in range(NS):
                    psv = pp1.tile([128, C], f32, name="psv", tag="ps_qk")
                    for kt in range(KD):
                        nc.tensor.matmul(
                            psv[:],
                            lhsT=xT_sb[kt][:, st * 128:(st + 1) * 128],
                            rhs=w_sb["v"][kt][:],
                            start=(kt == 0), stop=(kt == KD - 1),
                        )
                    nc.vector.tensor_add(
                        v_sb[:, st, :, 0:A],
                        psv[:].rearrange("p (h a) -> p h a", a=A),
                        bvb_sb[:].rearrange("p (h a) -> p h a", a=A),
                    )

            def attention_pair(p):
                # Heads 2p/2p+1 in row-halves 0:64/64:128 of tile p; adjacent
                # S^T matmuls use disjoint PE row groups (run concurrently).
                # Per kt-group: S^T (PE) -> exp (ACT) -> AV accumulate (PE).
                heads = (2 * p, 2 * p + 1)
                for qc in range(QC):
                    qs = slice(qc * QW, (qc + 1) * QW)
                    avs = [avp.tile([A + 1, QW], f32, name=f"av{hh}",
                                    tag=f"av{hh}") for hh in range(2)]
                    for ng in range(NG):
                        pts = []
                        for hh in range(2):
                            off = hh * A
                            st_ = stp.tile([128, KT_PER_ST * QW], f32,
                                           name=f"st{hh}", tag=f"st{hh}")
                            for jj in range(KT_PER_ST):
                                kt = ng * KT_PER_ST + jj
                                nc.tensor.matmul(
                                    st_[:, jj * QW:(jj + 1) * QW],
                                    lhsT=kT_sb[p][off:off + A,
                                                  kt * 128:(kt + 1) * 128],
                                    rhs=qT_sb[p][off:off + A, qs],
                                    start=True, stop=True,
                                )
                            pt = ptpool.tile([128, KT_PER_ST * QW], f32r,
                                             name=f"pt{hh}", tag=f"pt{hh}")
                            nc.scalar.activation(pt[:], st_[:], AF.Exp,
                                                 scale=0.125)
                            pts.append(pt)
                        for jj in range(KT_PER_ST):
                            kt = ng * KT_PER_ST + jj
                            for hh in range(2):
                                nc.tensor.matmul(
                                    avs[hh][:],
                                    lhsT=v_sb[:, kt, heads[hh], :],
                                    rhs=pts[hh][:, jj * QW:(jj + 1) * QW],
                                    start=(kt == 0), stop=(kt == NS - 1),
                                )
                    # normalize: attn[a, q] = av[a, q] / av[A, q]
                    for hh in range(2):
                        av = avs[hh]
                        off = hh * A
                        rec_f = spool.tile([1, QW], f32, name="rec_f",
                                           tag="rec_f")
                        nc.vector.reciprocal(rec_f[:], av[A:A + 1, :])
                        rec = spool.tile([1, QW], f32r, name="rec", tag="rec")
                        nc.vector.tensor_copy(rec[:], rec_f[:])
                        bc = pp1.tile([A, QW], f32, name="bc", tag="ps_qk")
                        nc.tensor.matmul(bc[:], lhsT=ones_sb[:], rhs=rec[:],
                                         start=True, stop=True)
                        bcs = spool.tile([A, QW], f32, name="bcs", tag="bcs")
                        nc.vector.tensor_copy(bcs[:], bc[:])
                        nc.vector.tensor_mul(attn_sb[p][off:off + A, qs],
                                             av[0:A, :], bcs[:])

            a2a_out = []

            def a2a_ship(t2):
                # ship attn tile t2 (heads 2*t2, 2*t2+1) to all peers
                a_in = dpool.tile([n_cores, 128, sq], f32r,
                                  name=f"a2a_in{t2}", uniquify=False)
                a_out = dpool.tile([n_cores, 128, sq], f32r,
                                   name=f"a2a_out{t2}", uniquify=False)
                for bb in range(n_cores // GROUPS):
                    for g2 in range(GROUPS):
                        nc.sync.dma_start(
                            a_in[bb * GROUPS + g2, :, :],
                            attn_sb[t2][:, g2 * sq:(g2 + 1) * sq])
                nc.gpsimd.collective_compute(
                    "AllToAll", mybir.AluOpType.bypass,
                    replica_groups=[list(range(n_cores))],
                    ins=[a_in.opt()], outs=[a_out.opt()],
                )
                a2a_out.append(a_out)

            # ---------------- schedule ----------------
            project_qk(0)
            project_v()
            attention_pair(0)
            if mode == "a2a":
                a2a_ship(0)
            project_qk(1)     # PE filler while attention 0 drains ACT
            attention_pair(1)
            if mode == "a2a":
                a2a_ship(1)

            # ---------------- output projection ----------------
            if mode == "a2a":
                for bb, outx_d in ((0, out0_d), (1, out1_d)):
                    # ktile order: all t2=0 shards first, so fc accumulation
                    # can start after AllToAll #1 while #2 is still in flight
                    pairs = [(g2, t2) for t2 in range(MC)
                             for g2 in range(GROUPS)]
                    fc_sb = []
                    for g2, t2 in pairs:
                        fct = fcpool.tile([128, sq], f32r,
                                          name=f"fc{bb}_{g2}_{t2}",
                                          tag=f"fc{bb}_{g2}_{t2}")
                        if cdt == "bf16":
                            nc.sync.dma_start(
                                fct[:], a2a_out[t2][bb * GROUPS + g2, :, :])
                        else:
                            nc.gpsimd.dma_start(
                                fct[:], a2a_out[t2][bb * GROUPS + g2, :, :])
                        fc_sb.append(fct)
                    wo_of = [wo_sb[2 * g2 + t2] for g2, t2 in pairs]
                    for mt in range(sq // 128):
                        ob = opool.tile([128, d], f32, name="ob", tag="ob")
                        for nn in range(d // OW):
                            ns_ = slice(nn * OW, (nn + 1) * OW)
                            ps = pp1.tile([128, OW], f32, name="ps_o",
                                          tag="ps_qk")
                            for kt in range(len(fc_sb)):
                                nc.tensor.matmul(
                                    ps[:],
                                    lhsT=fc_sb[kt][:, mt * 128:(mt + 1) * 128],
                                    rhs=wo_of[kt][:, ns_],
                                    start=(kt == 0),
                                    stop=(kt == len(fc_sb) - 1),
                                )
                            nc.vector.tensor_add(ob[:, ns_], ps[:],
                                                 bob_sb[:, ns_])
                        nc.sync.dma_start(
                            outx_d[mt * 128:(mt + 1) * 128, :], ob[:])
            else:
                for mt in range(NS):
                    ob = opool.tile([128, d], f32, name="ob", tag="ob")
                    for nn in range(d // OW):
                        ns_ = slice(nn * OW, (nn + 1) * OW)
                        ps = pp1.tile([128, OW], f32, name="ps_o",
                                      tag="ps_qk")
                        for kt in range(MC):
                            nc.tensor.matmul(
                                ps[:],
                                lhsT=attn_sb[kt][:, mt * 128:(mt + 1) * 128],
                                rhs=wo_sb[kt][:, ns_],
                                start=(kt == 0), stop=(kt == MC - 1),
                            )
                        nc.vector.tensor_add(ob[:, ns_], ps[:],
                                             bob_sb[:, ns_])
                    nc.sync.dma_start(out_d[mt * 128:(mt + 1) * 128, :],
                                      ob[:])

    nc.compile()
    return nc


def wo_sb_g(wo_sb, kt):
    # host mode: contraction is only over this core's C rows of Wo; the host
    # passes the [C, d] slice in "wo" (padded tile list indexed 0..MC-1)
    return wo_sb[kt]


def make_in_maps(x, Wq, bq, Wk, bk, Wv, bv, Wo, bo, n_cores=N_CORES, mode=MODE,
                 cdt=None):
    cdt = cdt or CDT
    d = x.shape[2]
    MC = C // 128
    f = np.float32
    if cdt == "bf16":
        import ml_dtypes
        cf = ml_dtypes.bfloat16
    else:
        cf = np.float32
    in_maps = []
    for core in range(n_cores):
        b, g = divmod(core, GROUPS)
        cs = slice(g * C, (g + 1) * C)
        m = {
            "xT": np.ascontiguousarray(x[b].T.astype(cf)),
            "wq": np.ascontiguousarray(Wq[:, cs].astype(cf)),
            "wk": np.ascontiguousarray(Wk[:, cs].astype(cf)),
            "wv": np.ascontiguousarray(Wv[:, cs].astype(cf)),
            "bqs": np.ascontiguousarray(bq[cs].reshape(MC, 128).T, dtype=f),
            "bks": np.ascontiguousarray(bk[cs].reshape(MC, 128).T, dtype=f),
            "bvb": np.ascontiguousarray(np.broadcast_to(bv[cs], (128, C)), dtype=f),
        }
        if mode == "a2a":
            m["wo"] = np.ascontiguousarray(Wo.astype(cf))
            m["bob"] = np.ascontiguousarray(np.broadcast_to(bo, (128, d)), dtype=f)
        else:
            m["wo"] = np.ascontiguousarray(Wo[cs].astype(cf))
            bob = np.broadcast_to(bo, (128, d)).astype(f) if g == 0 else \
                np.zeros((128, d), f)
            m["bob"] = np.ascontiguousarray(bob)
        in_maps.append(m)
    return in_maps


_nc_cache = {}


def _get_nc(mode=MODE):
    key = ("nc", mode)
    if key not in _nc_cache:
        _nc_cache[key] = build_nc(mode=mode)
    return _nc_cache[key]


def assemble(results, mode=MODE):
    out = np.empty((B, S, D), np.float32)
    if mode == "a2a":
        for core in range(N_CORES):
            b, g = divmod(core, GROUPS)
            out[b, g * SQ:(g + 1) * SQ, :] = results[core][f"out{b}"]
    else:
        for b in range(B):
            acc = results[b * GROUPS]["out"].copy()
            for g in range(1, GROUPS):
                acc += results[b * GROUPS + g]["out"]
            out[b] = acc
    return out


def kernel(x, Wq, bq, Wk, bk, Wv, bv, Wo, bo, _trace=False, _mode=None):
    from concourse.bass_utils import run_bass_kernel_spmd

    mode = _mode or MODE
    nc = _get_nc(mode)
    in_maps = make_in_maps(x, Wq, bq, Wk, bk, Wv, bv, Wo, bo, mode=mode)
    res = run_bass_kernel_spmd(nc, in_maps, core_ids=list(range(N_CORES)),
                               trace=_trace)
    _nc_cache["last_result"] = res
    return assemble(res.results, mode=mode)



# revision 8
# speedup vs baseline: 1.1691x; 1.1691x over previous
"""Multi-head attention (B=2,S=2048,D=1024,H=16,A=64) on 8 trn2 NeuronCores.

Sharding: core = 4*b + g  (b = batch, g = head-group of 4 heads).
Per core, feature-on-partition layout throughout:
  qT,kT = matmuls of Wq/Wk vs xT;  v natural; S^T per head; softmax over keys
  via exp (no max-sub; scores ~ N(0,1)) with the denominator produced by a
  ones-column appended to v; normalized attT [C=256, S] per core.
  fc_out: each core computes its partial over the full sequence of its batch;
  the host sums the 4 partials per batch.

Schedule: software-pipelined units u=(pair, qc). Per block, AV+normalize of
unit i-1 overlaps the exp stream of unit i on ACT; projection work (v, qk of
tile 1) and fc_out chunks fill PE slack inside the ACT-bound stretches.
Softmax normalize = reciprocal_approx_fast (DVE) + partition_broadcast
(GpSimd) + one DVE multiply; per-qc fc_out spreads the output DMA.
"""

from collections import deque

import numpy as np

B, S, D, H, A = 2, 2048, 1024, 16, 64
GROUPS = 4              # head groups (cores per batch)
HPG = H // GROUPS       # heads per core = 4
C = HPG * A             # channels per core = 256
N_CORES = 8
SQ = S // GROUPS


def build_nc(s=S, d=D, n_cores=N_CORES):
    import concourse.bass as bass
    import concourse.mybir as mybir
    import concourse.tile as tile
    from concourse import bacc

    f32 = mybir.dt.float32
    bf16 = mybir.dt.bfloat16
    AF = mybir.ActivationFunctionType

    KD = d // 128        # d-tiles (contraction for projections) = 8
    MC = C // 128        # c-tiles per core = 2 (pairs of heads)
    NS = s // 128        # seq tiles = 16
    QC = s // 512        # q chunks = 4
    QW = 512             # q chunk width
    KT_PER_ST = 2        # k-tiles packed per st/pt tile (exp batching)
    NG = NS // KT_PER_ST # st/pt groups per (p, qc) = 8
    OW = 512             # output free-dim chunk

    nc = bacc.Bacc(
        "TRN2", target_bir_lowering=False, debug=False,
        enable_asserts=True, num_devices=n_cores,
    )

    xT_d = nc.dram_tensor("xT", [d, s], bf16, kind="ExternalInput").ap()
    wq_d = nc.dram_tensor("wq", [d, C], bf16, kind="ExternalInput").ap()
    wk_d = nc.dram_tensor("wk", [d, C], bf16, kind="ExternalInput").ap()
    wv_d = nc.dram_tensor("wv", [d, C], bf16, kind="ExternalInput").ap()
    wo_d = nc.dram_tensor("wo", [C, d], bf16, kind="ExternalInput").ap()
    bqs_d = nc.dram_tensor("bqs", [128, MC], f32, kind="ExternalInput").ap()
    bks_d = nc.dram_tensor("bks", [128, MC], f32, kind="ExternalInput").ap()
    bvb_d = nc.dram_tensor("bvb", [128, C], f32, kind="ExternalInput").ap()
    bob_d = nc.dram_tensor("bob", [128, d], f32, kind="ExternalInput").ap()
    out_d = nc.dram_tensor("out", [s, d], f32, kind="ExternalOutput").ap()

    with tile.TileContext(nc) as tc:
        with tc.tile_pool(name="const", bufs=1) as cpool, \
             tc.tile_pool(name="qkv", bufs=1) as qpool, \
             tc.tile_pool(name="wop", bufs=1) as wopool, \
             tc.tile_pool(name="xTw", bufs=1) as xpool, \
             tc.tile_pool(name="ptp", bufs=10) as ptpool, \
             tc.tile_pool(name="sml", bufs=3) as spool, \
             tc.tile_pool(name="osb", bufs=3) as opool, \
             tc.tile_pool(name="ps1", bufs=2, space="PSUM") as pp1, \
             tc.tile_pool(name="pst", bufs=1, space="PSUM") as stp, \
             tc.tile_pool(name="pav", bufs=1, space="PSUM") as avp:

            # warm the ACT exp table set while input DMAs are in flight
            warm_f = cpool.tile([1, 16], f32, name="warm_f")
            nc.vector.memset(warm_f[:], 0.0)
            warm_o = cpool.tile([1, 16], f32, name="warm_o")
            nc.scalar.activation(warm_o[:], warm_f[:], AF.Exp, scale=1.0)

            ones_f = cpool.tile([1, A], f32, name="ones_f")
            nc.vector.memset(ones_f[:], 1.0)
            ones_sb = cpool.tile([1, A], bf16, name="ones_sb")
            nc.scalar.copy(ones_sb[:], ones_f[:])

            bq_sb = cpool.tile([128, MC], f32, name="bq_sb")
            bk_sb = cpool.tile([128, MC], f32, name="bk_sb")
            bvb_sb = cpool.tile([128, C], f32, name="bvb_sb")
            bob_sb = cpool.tile([128, d], f32, name="bob_sb")

            qT_sb = [qpool.tile([128, s], bf16, name=f"qT{mt}", tag=f"qT{mt}")
                     for mt in range(MC)]
            kT_sb = [qpool.tile([128, s], bf16, name=f"kT{mt}", tag=f"kT{mt}")
                     for mt in range(MC)]
            # v, padded per head with a ones column: [128, NS, HPG, A+1]
            v_sb = qpool.tile([128, NS, HPG, A + 1], bf16, name="v_sb", tag="v")
            vones_f = cpool.tile([128, NS * HPG], f32, name="vones_f")
            nc.vector.memset(vones_f[:], 1.0)
            nc.vector.tensor_copy(
                v_sb[:, :, :, A],
                vones_f[:].rearrange("p (t h) -> p t h", h=HPG))

            wo_sb = [wopool.tile([128, d], bf16, name=f"wo{kt}", tag=f"wo{kt}")
                     for kt in range(MC)]
            attn_sb = [qpool.tile([128, s], bf16, name=f"attn{t}", tag=f"at{t}")
                       for t in range(MC)]

            # ---------------- input loads ----------------
            xT_sb = [xpool.tile([128, s], bf16, name=f"xT{kt}", tag=f"x{kt}")
                     for kt in range(KD)]
            w_sb = {}
            for wname in ("q", "k", "v"):
                w_sb[wname] = [
                    xpool.tile([128, C], bf16, name=f"w{wname}{kt}",
                               tag=f"w{wname}{kt}")
                    for kt in range(KD)
                ]
            ldeng = [nc.sync, nc.gpsimd]
            li = 0

            def load(dst, src_ap):
                nonlocal li
                ldeng[li % 2].dma_start(dst, src_ap)
                li += 1

            wds = {"q": wq_d, "k": wk_d, "v": wv_d}
            for kt in range(KD):
                ks = slice(kt * 128, (kt + 1) * 128)
                load(xT_sb[kt][:], xT_d[ks, :])
                load(w_sb["k"][kt][:], wds["k"][ks, :])
            for kt in range(KD):
                ks = slice(kt * 128, (kt + 1) * 128)
                load(w_sb["q"][kt][:], wds["q"][ks, :])
                load(w_sb["v"][kt][:], wds["v"][ks, :])
            load(bq_sb[:], bqs_d[:, :])
            load(bk_sb[:], bks_d[:, :])
            load(bvb_sb[:], bvb_d[:, :])
            for kt in range(MC):
                load(wo_sb[kt][:], wo_d[kt * 128:(kt + 1) * 128, :])
            load(bob_sb[:], bob_d[:, :])

            # ---------------- building blocks ----------------
            def proj_qk_group(wname, mt, qc):
                # qT/kT[c, qs] = sum_d W[d, c] * xT[d, qs], bias via DVE
                dst, bias = ((qT_sb, bq_sb) if wname == "q" else
                             (kT_sb, bk_sb))
                qs = slice(qc * QW, (qc + 1) * QW)
                ps = pp1.tile([128, QW], f32, name="ps_qk", tag="ps_qk")
                for kt in range(KD):
                    nc.tensor.matmul(
                        ps[:],
                        lhsT=w_sb[wname][kt][:, mt * 128:(mt + 1) * 128],
                        rhs=xT_sb[kt][:, qs],
                        start=(kt == 0), stop=(kt == KD - 1),
                    )
                nc.vector.tensor_scalar_add(
                    dst[mt][:, qs], ps[:], bias[:, mt:mt + 1])

            def proj_v_group(st):
                # v[s_tile, c] = sum_d xT[d, s_tile] * Wv[d, c]
                psv = pp1.tile([128, C], f32, name="psv", tag="ps_qk")
                for kt in range(KD):
                    nc.tensor.matmul(
                        psv[:],
                        lhsT=xT_sb[kt][:, st * 128:(st + 1) * 128],
                        rhs=w_sb["v"][kt][:],
                        start=(kt == 0), stop=(kt == KD - 1),
                    )
                nc.vector.tensor_add(
                    v_sb[:, st, :, 0:A],
                    psv[:].rearrange("p (h a) -> p h a", a=A),
                    bvb_sb[:].rearrange("p (h a) -> p h a", a=A),
                )

            def scores_group(p, qc, ng):
                # S^T for heads 2p (rows 0:64) / 2p+1 (rows 64:128); the two
                # K=64 matmuls hit disjoint PE row groups and co-run.
                # exp issued immediately after each head's scores.
                qs = slice(qc * QW, (qc + 1) * QW)
                pts = []
                for hh in range(2):
                    off = hh * A
                    st_ = stp.tile([128, KT_PER_ST * QW], f32,
                                   name=f"st{hh}", tag=f"st{hh}")
                    for jj in range(KT_PER_ST):
                        kt = ng * KT_PER_ST + jj
                        nc.tensor.matmul(
                            st_[:, jj * QW:(jj + 1) * QW],
                            lhsT=kT_sb[p][off:off + A,
                                          kt * 128:(kt + 1) * 128],
                            rhs=qT_sb[p][off:off + A, qs],
                            start=True, stop=True,
                        )
                    pt = ptpool.tile([128, KT_PER_ST * QW], bf16,
                                     name=f"pt{hh}", tag=f"pt{hh}")
                    nc.scalar.activation(pt[:], st_[:], AF.Exp, scale=0.125)
                    pts.append(pt)
                return pts

            def av_group(p, ng, avs, pts):
                heads = (2 * p, 2 * p + 1)
                for jj in range(KT_PER_ST):
                    kt = ng * KT_PER_ST + jj
                    for hh in range(2):
                        nc.tensor.matmul(
                            avs[hh][:],
                            lhsT=v_sb[:, kt, heads[hh], :],
                            rhs=pts[hh][:, jj * QW:(jj + 1) * QW],
                            start=(kt == 0), stop=(kt == NS - 1),
                        )

            def norm_unit(p, qc, avs):
                # attn[a, q] = av[a, q] * (1 / av[A, q]); recip on DVE,
                # column-broadcast on GpSimd, one DVE multiply.
                qs = slice(qc * QW, (qc + 1) * QW)
                for hh in range(2):
                    av = avs[hh]
                    off = hh * A
                    den = spool.tile([1, QW], f32, name="den", tag="den")
                    nc.vector.tensor_copy(den[:], av[A:A + 1, :])
                    rec = spool.tile([1, QW], f32, name="rec", tag="rec")
                    nc.vector.reciprocal_approx_fast(rec[:], den[:])
                    recb = spool.tile([1, QW], bf16, name="recb", tag="recb")
                    nc.vector.tensor_copy(recb[:], rec[:])
                    bc = pp1.tile([A, QW], f32, name="bc", tag="ps_qk")
                    nc.tensor.matmul(bc[:], lhsT=ones_sb[:], rhs=recb[:],
                                     start=True, stop=True)
                    bcs = spool.tile([A, QW], f32, name="bcs", tag="bcd")
                    nc.vector.tensor_copy(bcs[:], bc[:])
                    nc.vector.tensor_mul(
                        attn_sb[p][off:off + A, qs], av[0:A, :], bcs[:])

            def fc_group(mt):
                # out rows [mt*128, (mt+1)*128) = attT^T @ Wo + bo
                ob = opool.tile([128, d], f32, name="ob", tag="ob")
                for nn in range(d // OW):
                    ns_ = slice(nn * OW, (nn + 1) * OW)
                    ps = pp1.tile([128, OW], f32, name="ps_o", tag="ps_qk")
                    for kt in range(MC):
                        nc.tensor.matmul(
                            ps[:],
                            lhsT=attn_sb[kt][:, mt * 128:(mt + 1) * 128],
                            rhs=wo_sb[kt][:, ns_],
                            start=(kt == 0), stop=(kt == MC - 1),
                        )
                    nc.vector.tensor_add(ob[:, ns_], ps[:], bob_sb[:, ns_])
                nc.sync.dma_start(out_d[mt * 128:(mt + 1) * 128, :], ob[:])

            # ---------------- pipelined schedule ----------------
            # filler: PE work drained into slack inside ACT-bound stretches
            filler = deque()
            done = set()

            def push(key, fn):
                filler.append((key, fn))

            def drain(n=1):
                for _ in range(n):
                    if not filler:
                        return
                    key, fn = filler.popleft()
                    fn()
                    done.add(key)

            def drain_until(key):
                while key not in done and filler:
                    k, fn = filler.popleft()
                    fn()
                    done.add(k)

            for st in range(4):
                push(("v", st), (lambda st=st: proj_v_group(st)))
            push(("qT", 0, 1), (lambda: proj_qk_group("q", 0, 1)))
            for st in range(4, 8):
                push(("v", st), (lambda st=st: proj_v_group(st)))
            push(("qT", 0, 2), (lambda: proj_qk_group("q", 0, 2)))
            for st in range(8, 12):
                push(("v", st), (lambda st=st: proj_v_group(st)))
            push(("qT", 0, 3), (lambda: proj_qk_group("q", 0, 3)))
            for st in range(12, 16):
                push(("v", st), (lambda st=st: proj_v_group(st)))
            for qc in range(QC):
                push(("kT", 1, qc), (lambda qc=qc: proj_qk_group("k", 1, qc)))
            for qc in range(QC):
                push(("qT", 1, qc), (lambda qc=qc: proj_qk_group("q", 1, qc)))

            units = [(0, 0), (0, 1), (0, 2), (1, 0), (0, 3), (1, 1),
                     (1, 2), (1, 3)]
            # fc chunk qc becomes ready once (1, qc) is normalized
            fc_ready_after = {(1, qc): qc for qc in range(QC)}

            # prologue: kT(0) chunk 0, qT(0) chunk 0
            proj_qk_group("k", 0, 0)
            done.add(("kT", 0, 0))
            proj_qk_group("q", 0, 0)
            done.add(("qT", 0, 0))

            prev = None          # (p, qc, avs, pts_list)
            for i, (p, qc) in enumerate(units):
                # AV + normalize of previous unit (overlaps this unit's exps)
                if prev is not None:
                    pp_, pqc_, pavs_, ppts_ = prev
                    for ng in range(NG):
                        for st in range(2 * ng + 1, -1, -1):
                            if ("v", st) not in done:
                                drain_until(("v", st))
                                break
                        av_group(pp_, ng, pavs_, ppts_[ng])
                        drain(1)
                    norm_unit(pp_, pqc_, pavs_)
                    if (pp_, pqc_) in fc_ready_after:
                        fqc = fc_ready_after[(pp_, pqc_)]
                        for mt in range(4 * fqc, 4 * fqc + 4):
                            filler.appendleft(
                                (("fc", mt), (lambda mt=mt: fc_group(mt))))

                # scores + exp stream of this unit
                if p == 1:
                    drain_until(("kT", 1, QC - 1))
                    drain_until(("qT", 1, qc))
                else:
                    drain_until(("qT", 0, qc))
                avs = [avp.tile([A + 1, QW], f32, name=f"av{hh}",
                                tag=f"av{hh}") for hh in range(2)]
                pts_list = []
                for ng in range(NG):
                    need_kt = ("kT", p, min(QC - 1, (ng * KT_PER_ST + 1) // 4))
                    if p == 0 and need_kt not in done and i == 0:
                        # prologue path: issue the kT chunk directly
                        proj_qk_group("k", 0, need_kt[2])
                        done.add(need_kt)
                    pts_list.append(scores_group(p, qc, ng))
                    drain(1)
                prev = (p, qc, avs, pts_list)

            # tail: AV + normalize + fc of the last unit
            pp_, pqc_, pavs_, ppts_ = prev
            for ng in range(NG):
                av_group(pp_, ng, pavs_, ppts_[ng])
            norm_unit(pp_, pqc_, pavs_)
            while filler:
                drain(1)
            for mt in range(4 * pqc_, 4 * pqc_ + 4):
                fc_group(mt)

    nc.compile()
    return nc


def make_in_maps(x, Wq, bq, Wk, bk, Wv, bv, Wo, bo, n_cores=N_CORES):
    import ml_dtypes
    cf = ml_dtypes.bfloat16
    d = x.shape[2]
    MC = C // 128
    f = np.float32
    in_maps = []
    for core in range(n_cores):
        b, g = divmod(core, GROUPS)
        cs = slice(g * C, (g + 1) * C)
        bob = np.broadcast_to(bo, (128, d)).astype(f) if g == 0 else \
            np.zeros((128, d), f)
        m = {
            "xT": np.ascontiguousarray(x[b].T.astype(cf)),
            "wq": np.ascontiguousarray(Wq[:, cs].astype(cf)),
            "wk": np.ascontiguousarray(Wk[:, cs].astype(cf)),
            "wv": np.ascontiguousarray(Wv[:, cs].astype(cf)),
            "wo": np.ascontiguousarray(Wo[cs].astype(cf)),
            "bqs": np.ascontiguousarray(bq[cs].reshape(MC, 128).T, dtype=f),
            "bks": np.ascontiguousarray(bk[cs].reshape(MC, 128).T, dtype=f),
            "bvb": np.ascontiguousarray(np.broadcast_to(bv[cs], (128, C)), dtype=f),
            "bob": np.ascontiguousarray(bob),
        }
        in_maps.append(m)
    return in_maps


_nc_cache = {}


def _get_nc():
    if "nc" not in _nc_cache:
        _nc_cache["nc"] = build_nc()
    return _nc_cache["nc"]


def assemble(results):
    out = np.empty((B, S, D), np.float32)
    for b in range(B):
        acc = results[b * GROUPS]["out"].copy()
        for g in range(1, GROUPS):
            acc += results[b * GROUPS + g]["out"]
        out[b] = acc
    return out


def kernel(x, Wq, bq, Wk, bk, Wv, bv, Wo, bo, _trace=False, _mode=None):
    from concourse.bass_utils import run_bass_kernel_spmd

    nc = _get_nc()
    in_maps = make_in_maps(x, Wq, bq, Wk, bk, Wv, bv, Wo, bo)
    res = run_bass_kernel_spmd(nc, in_maps, core_ids=list(range(N_CORES)),
                               trace=_trace)
    _nc_cache["last_result"] = res
    return assemble(res.results)


# revision 10
# speedup vs baseline: 1.2045x; 1.0303x over previous
"""Multi-head attention (B=2,S=2048,D=1024,H=16,A=64) on 8 trn2 NeuronCores.

Sharding: core = 4*b + g  (b = batch, g = head-group of 4 heads).
Per core, feature-on-partition layout throughout:
  qT,kT = matmuls of Wq/Wk vs xT;  v natural; S^T per head; softmax over keys
  via exp (no max-sub; scores ~ N(0,1)) with the denominator produced by a
  ones-column appended to v; normalized attT [C=256, S] per core.
  fc_out: each core computes its partial over the full sequence of its batch;
  the host sums the 4 partials per batch.

Schedule: software-pipelined units u=(pair, qc). Per block, AV+normalize of
unit i-1 overlaps the exp stream of unit i on ACT; projection work (v, qk of
tile 1) and fc_out chunks fill PE slack inside the ACT-bound stretches.
Softmax normalize = reciprocal_approx_fast (DVE) + partition_broadcast
(GpSimd) + one DVE multiply; per-qc fc_out spreads the output DMA.
"""

from collections import deque

import numpy as np

B, S, D, H, A = 2, 2048, 1024, 16, 64
GROUPS = 4              # head groups (cores per batch)
HPG = H // GROUPS       # heads per core = 4
C = HPG * A             # channels per core = 256
N_CORES = 8
SQ = S // GROUPS


def build_nc(s=S, d=D, n_cores=N_CORES):
    import concourse.bass as bass
    import concourse.mybir as mybir
    import concourse.tile as tile
    from concourse import bacc

    f32 = mybir.dt.float32
    bf16 = mybir.dt.bfloat16
    AF = mybir.ActivationFunctionType

    KD = d // 128        # d-tiles (contraction for projections) = 8
    MC = C // 128        # c-tiles per core = 2 (pairs of heads)
    NS = s // 128        # seq tiles = 16
    QC = s // 512        # q chunks = 4
    QW = 512             # q chunk width
    KT_PER_ST = 2        # k-tiles packed per st/pt tile (exp batching)
    NG = NS // KT_PER_ST # st/pt groups per (p, qc) = 8
    OW = 512             # output free-dim chunk

    nc = bacc.Bacc(
        "TRN2", target_bir_lowering=False, debug=False,
        enable_asserts=True, num_devices=n_cores,
    )

    xT_d = nc.dram_tensor("xT", [d, s], bf16, kind="ExternalInput").ap()
    wq_d = nc.dram_tensor("wq", [d, C], bf16, kind="ExternalInput").ap()
    wk_d = nc.dram_tensor("wk", [d, C], bf16, kind="ExternalInput").ap()
    wv_d = nc.dram_tensor("wv", [d, C], bf16, kind="ExternalInput").ap()
    wo_d = nc.dram_tensor("wo", [C, d], bf16, kind="ExternalInput").ap()
    bqs_d = nc.dram_tensor("bqs", [128, MC], f32, kind="ExternalInput").ap()
    bks_d = nc.dram_tensor("bks", [128, MC], f32, kind="ExternalInput").ap()
    bvb_d = nc.dram_tensor("bvb", [128, C], f32, kind="ExternalInput").ap()
    bob_d = nc.dram_tensor("bob", [128, d], f32, kind="ExternalInput").ap()
    out_d = nc.dram_tensor("out", [s, d], f32, kind="ExternalOutput").ap()

    with tile.TileContext(nc) as tc:
        with tc.tile_pool(name="const", bufs=1) as cpool, \
             tc.tile_pool(name="qkv", bufs=1) as qpool, \
             tc.tile_pool(name="wop", bufs=1) as wopool, \
             tc.tile_pool(name="xTw", bufs=1) as xpool, \
             tc.tile_pool(name="ptp", bufs=10) as ptpool, \
             tc.tile_pool(name="sml", bufs=3) as spool, \
             tc.tile_pool(name="osb", bufs=3) as opool, \
             tc.tile_pool(name="ps1", bufs=2, space="PSUM") as pp1, \
             tc.tile_pool(name="pst", bufs=1, space="PSUM") as stp, \
             tc.tile_pool(name="pav", bufs=1, space="PSUM") as avp:

            # warm the ACT exp table set while input DMAs are in flight
            warm_f = cpool.tile([1, 16], f32, name="warm_f")
            nc.vector.memset(warm_f[:], 0.0)
            warm_o = cpool.tile([1, 16], f32, name="warm_o")
            nc.scalar.activation(warm_o[:], warm_f[:], AF.Exp, scale=1.0)

            ones_f = cpool.tile([1, A], f32, name="ones_f")
            nc.vector.memset(ones_f[:], 1.0)
            ones_sb = cpool.tile([1, A], bf16, name="ones_sb")
            nc.scalar.copy(ones_sb[:], ones_f[:])

            bq_sb = cpool.tile([128, MC], f32, name="bq_sb")
            bk_sb = cpool.tile([128, MC], f32, name="bk_sb")
            bvb_sb = cpool.tile([128, C], f32, name="bvb_sb")
            bob_sb = cpool.tile([128, d], f32, name="bob_sb")

            qT_sb = [qpool.tile([128, s], bf16, name=f"qT{mt}", tag=f"qT{mt}")
                     for mt in range(MC)]
            kT_sb = [qpool.tile([128, s], bf16, name=f"kT{mt}", tag=f"kT{mt}")
                     for mt in range(MC)]
            # v, padded per head with a ones column: [128, NS, HPG, A+1]
            v_sb = qpool.tile([128, NS, HPG, A + 1], bf16, name="v_sb", tag="v")
            vones_f = cpool.tile([128, NS * HPG], f32, name="vones_f")
            nc.vector.memset(vones_f[:], 1.0)
            nc.vector.tensor_copy(
                v_sb[:, :, :, A],
                vones_f[:].rearrange("p (t h) -> p t h", h=HPG))

            wo_sb = [wopool.tile([128, d], bf16, name=f"wo{kt}", tag=f"wo{kt}")
                     for kt in range(MC)]
            attn_sb = [qpool.tile([128, s], bf16, name=f"attn{t}", tag=f"at{t}")
                       for t in range(MC)]

            # ---------------- input loads ----------------
            xT_sb = [xpool.tile([128, s], bf16, name=f"xT{kt}", tag=f"x{kt}")
                     for kt in range(KD)]
            w_sb = {}
            for wname in ("q", "k", "v"):
                w_sb[wname] = [
                    xpool.tile([128, C], bf16, name=f"w{wname}{kt}",
                               tag=f"w{wname}{kt}")
                    for kt in range(KD)
                ]
            ldeng = [nc.sync, nc.gpsimd]
            li = 0

            def load(dst, src_ap):
                nonlocal li
                ldeng[li % 2].dma_start(dst, src_ap)
                li += 1

            wds = {"q": wq_d, "k": wk_d, "v": wv_d}
            for kt in range(KD):
                ks = slice(kt * 128, (kt + 1) * 128)
                load(xT_sb[kt][:], xT_d[ks, :])
                load(w_sb["k"][kt][:], wds["k"][ks, :])
            for kt in range(KD):
                ks = slice(kt * 128, (kt + 1) * 128)
                load(w_sb["q"][kt][:], wds["q"][ks, :])
                load(w_sb["v"][kt][:], wds["v"][ks, :])
            load(bq_sb[:], bqs_d[:, :])
            load(bk_sb[:], bks_d[:, :])
            load(bvb_sb[:], bvb_d[:, :])
            for kt in range(MC):
                load(wo_sb[kt][:], wo_d[kt * 128:(kt + 1) * 128, :])
            load(bob_sb[:], bob_d[:, :])

            # ---------------- building blocks ----------------
            def proj_qk_group(wname, mt, qc):
                # qT/kT[c, qs] = sum_d W[d, c] * xT[d, qs], bias via DVE
                dst, bias = ((qT_sb, bq_sb) if wname == "q" else
                             (kT_sb, bk_sb))
                qs = slice(qc * QW, (qc + 1) * QW)
                ps = pp1.tile([128, QW], f32, name="ps_qk", tag="ps_qk")
                for kt in range(KD):
                    nc.tensor.matmul(
                        ps[:],
                        lhsT=w_sb[wname][kt][:, mt * 128:(mt + 1) * 128],
                        rhs=xT_sb[kt][:, qs],
                        start=(kt == 0), stop=(kt == KD - 1),
                    )
                nc.vector.tensor_scalar_add(
                    dst[mt][:, qs], ps[:], bias[:, mt:mt + 1])

            def proj_v_group(st):
                # v[s_tile, c] = sum_d xT[d, s_tile] * Wv[d, c]
                psv = pp1.tile([128, C], f32, name="psv", tag="ps_qk")
                for kt in range(KD):
                    nc.tensor.matmul(
                        psv[:],
                        lhsT=xT_sb[kt][:, st * 128:(st + 1) * 128],
                        rhs=w_sb["v"][kt][:],
                        start=(kt == 0), stop=(kt == KD - 1),
                    )
                nc.vector.tensor_add(
                    v_sb[:, st, :, 0:A],
                    psv[:].rearrange("p (h a) -> p h a", a=A),
                    bvb_sb[:].rearrange("p (h a) -> p h a", a=A),
                )

            def scores_group(p, qc, ng):
                # S^T for heads 2p (rows 0:64) / 2p+1 (rows 64:128); the two
                # K=64 matmuls hit disjoint PE row groups and co-run.
                # exp issued immediately after each head's scores.
                qs = slice(qc * QW, (qc + 1) * QW)
                sts = [stp.tile([128, KT_PER_ST * QW], f32,
                                name=f"st{hh}", tag=f"st{hh}")
                       for hh in range(2)]
                # strict T0/T8 alternation so adjacent matmuls always hit
                # disjoint PE row groups and co-run
                for jj in range(KT_PER_ST):
                    kt = ng * KT_PER_ST + jj
                    for hh in range(2):
                        off = hh * A
                        nc.tensor.matmul(
                            sts[hh][:, jj * QW:(jj + 1) * QW],
                            lhsT=kT_sb[p][off:off + A,
                                          kt * 128:(kt + 1) * 128],
                            rhs=qT_sb[p][off:off + A, qs],
                            start=True, stop=True,
                        )
                pts = []
                for hh in range(2):
                    pt = ptpool.tile([128, KT_PER_ST * QW], bf16,
                                     name=f"pt{hh}", tag=f"pt{hh}")
                    nc.scalar.activation(pt[:], sts[hh][:], AF.Exp,
                                         scale=0.125)
                    pts.append(pt)
                return pts

            def av_group(p, ng, avs, pts):
                heads = (2 * p, 2 * p + 1)
                for jj in range(KT_PER_ST):
                    kt = ng * KT_PER_ST + jj
                    for hh in range(2):
                        nc.tensor.matmul(
                            avs[hh][:],
                            lhsT=v_sb[:, kt, heads[hh], :],
                            rhs=pts[hh][:, jj * QW:(jj + 1) * QW],
                            start=(kt == 0), stop=(kt == NS - 1),
                        )

            def norm_unit(p, qc, avs):
                # attn[a, q] = av[a, q] * (1 / av[A, q]); recip on DVE,
                # column-broadcast on GpSimd, one DVE multiply.
                qs = slice(qc * QW, (qc + 1) * QW)
                for hh in range(2):
                    av = avs[hh]
                    off = hh * A
                    # custom-DVE recip must read SBUF (PSUM source gave
                    # garbage on HW) — copy the denominator row out first
                    den = spool.tile([1, QW], f32, name="den", tag="den")
                    nc.vector.tensor_copy(den[:], av[A:A + 1, :])
                    rec = spool.tile([1, QW], f32, name="rec", tag="rec")
                    nc.vector.reciprocal_approx_fast(rec[:], den[:])
                    bcd = spool.tile([A, QW], f32, name="bcd", tag="bcd")
                    nc.gpsimd.partition_broadcast(bcd[:], rec[:], channels=A)
                    nc.vector.tensor_mul(
                        attn_sb[p][off:off + A, qs], av[0:A, :], bcd[:])

            def fc_group(mt):
                # out rows [mt*128, (mt+1)*128) = attT^T @ Wo + bo
                ob = opool.tile([128, d], f32, name="ob", tag="ob")
                for nn in range(d // OW):
                    ns_ = slice(nn * OW, (nn + 1) * OW)
                    ps = pp1.tile([128, OW], f32, name="ps_o", tag="ps_qk")
                    for kt in range(MC):
                        nc.tensor.matmul(
                            ps[:],
                            lhsT=attn_sb[kt][:, mt * 128:(mt + 1) * 128],
                            rhs=wo_sb[kt][:, ns_],
                            start=(kt == 0), stop=(kt == MC - 1),
                        )
                    nc.vector.tensor_add(ob[:, ns_], ps[:], bob_sb[:, ns_])
                nc.sync.dma_start(out_d[mt * 128:(mt + 1) * 128, :], ob[:])

            # ---------------- pipelined schedule ----------------
            # filler: PE work drained into slack inside ACT-bound stretches
            filler = deque()
            done = set()

            def push(key, fn):
                filler.append((key, fn))

            def drain(n=1):
                for _ in range(n):
                    if not filler:
                        return
                    key, fn = filler.popleft()
                    fn()
                    done.add(key)

            def drain_until(key):
                while key not in done and filler:
                    k, fn = filler.popleft()
                    fn()
                    done.add(k)

            for st in range(4):
                push(("v", st), (lambda st=st: proj_v_group(st)))
            push(("qT", 0, 1), (lambda: proj_qk_group("q", 0, 1)))
            for st in range(4, 8):
                push(("v", st), (lambda st=st: proj_v_group(st)))
            push(("qT", 0, 2), (lambda: proj_qk_group("q", 0, 2)))
            for st in range(8, 12):
                push(("v", st), (lambda st=st: proj_v_group(st)))
            push(("qT", 0, 3), (lambda: proj_qk_group("q", 0, 3)))
            for st in range(12, 16):
                push(("v", st), (lambda st=st: proj_v_group(st)))
            for qc in range(QC):
                push(("kT", 1, qc), (lambda qc=qc: proj_qk_group("k", 1, qc)))
            for qc in range(QC):
                push(("qT", 1, qc), (lambda qc=qc: proj_qk_group("q", 1, qc)))

            units = [(0, 0), (0, 1), (0, 2), (1, 0), (0, 3), (1, 1),
                     (1, 2), (1, 3)]
            # fc chunk qc becomes ready once (1, qc) is normalized
            fc_ready_after = {(1, qc): qc for qc in range(QC)}

            # prologue: kT(0) chunk 0, qT(0) chunk 0
            proj_qk_group("k", 0, 0)
            done.add(("kT", 0, 0))
            proj_qk_group("q", 0, 0)
            done.add(("qT", 0, 0))

            prev = None          # (p, qc, avs, pts_list)
            for i, (p, qc) in enumerate(units):
                # AV + normalize of previous unit (overlaps this unit's exps)
                if prev is not None:
                    pp_, pqc_, pavs_, ppts_ = prev
                    for ng in range(NG):
                        for st in range(2 * ng + 1, -1, -1):
                            if ("v", st) not in done:
                                drain_until(("v", st))
                                break
                        av_group(pp_, ng, pavs_, ppts_[ng])
                        drain(1)
                    norm_unit(pp_, pqc_, pavs_)
                    if (pp_, pqc_) in fc_ready_after:
                        fqc = fc_ready_after[(pp_, pqc_)]
                        for mt in range(4 * fqc, 4 * fqc + 4):
                            filler.appendleft(
                                (("fc", mt), (lambda mt=mt: fc_group(mt))))

                # scores + exp stream of this unit
                if p == 1:
                    drain_until(("kT", 1, QC - 1))
                    drain_until(("qT", 1, qc))
                else:
                    drain_until(("qT", 0, qc))
                avs = [avp.tile([A + 1, QW], f32, name=f"av{hh}",
                                tag=f"av{hh}") for hh in range(2)]
                pts_list = []
                for ng in range(NG):
                    need_kt = ("kT", p, min(QC - 1, (ng * KT_PER_ST + 1) // 4))
                    if p == 0 and need_kt not in done and i == 0:
                        # prologue path: issue the kT chunk directly
                        proj_qk_group("k", 0, need_kt[2])
                        done.add(need_kt)
                    pts_list.append(scores_group(p, qc, ng))
                    drain(1)
                prev = (p, qc, avs, pts_list)

            # tail: AV + normalize + fc of the last unit
            pp_, pqc_, pavs_, ppts_ = prev
            for ng in range(NG):
                av_group(pp_, ng, pavs_, ppts_[ng])
            norm_unit(pp_, pqc_, pavs_)
            while filler:
                drain(1)
            for mt in range(4 * pqc_, 4 * pqc_ + 4):
                fc_group(mt)

    nc.compile()
    return nc


def make_in_maps(x, Wq, bq, Wk, bk, Wv, bv, Wo, bo, n_cores=N_CORES):
    import ml_dtypes
    cf = ml_dtypes.bfloat16
    d = x.shape[2]
    MC = C // 128
    f = np.float32
    in_maps = []
    for core in range(n_cores):
        b, g = divmod(core, GROUPS)
        cs = slice(g * C, (g + 1) * C)
        bob = np.broadcast_to(bo, (128, d)).astype(f) if g == 0 else \
            np.zeros((128, d), f)
        m = {
            "xT": np.ascontiguousarray(x[b].T.astype(cf)),
            "wq": np.ascontiguousarray(Wq[:, cs].astype(cf)),
            "wk": np.ascontiguousarray(Wk[:, cs].astype(cf)),
            "wv": np.ascontiguousarray(Wv[:, cs].astype(cf)),
            "wo": np.ascontiguousarray(Wo[cs].astype(cf)),
            "bqs": np.ascontiguousarray(bq[cs].reshape(MC, 128).T, dtype=f),
            "bks": np.ascontiguousarray(bk[cs].reshape(MC, 128).T, dtype=f),
            "bvb": np.ascontiguousarray(np.broadcast_to(bv[cs], (128, C)), dtype=f),
            "bob": np.ascontiguousarray(bob),
        }
        in_maps.append(m)
    return in_maps


_nc_cache = {}


def _get_nc():
    if "nc" not in _nc_cache:
        _nc_cache["nc"] = build_nc()
    return _nc_cache["nc"]


def assemble(results):
    out = np.empty((B, S, D), np.float32)
    for b in range(B):
        acc = results[b * GROUPS]["out"].copy()
        for g in range(1, GROUPS):
            acc += results[b * GROUPS + g]["out"]
        out[b] = acc
    return out


def kernel(x, Wq, bq, Wk, bk, Wv, bv, Wo, bo, _trace=False, _mode=None):
    from concourse.bass_utils import run_bass_kernel_spmd

    nc = _get_nc()
    in_maps = make_in_maps(x, Wq, bq, Wk, bk, Wv, bv, Wo, bo)
    res = run_bass_kernel_spmd(nc, in_maps, core_ids=list(range(N_CORES)),
                               trace=_trace)
    _nc_cache["last_result"] = res
    return assemble(res.results)


# revision 17
# speedup vs baseline: 1.2948x; 1.0750x over previous
"""Multi-head attention (B=2,S=2048,D=1024,H=16,A=64) on 8 trn2 NeuronCores.

Sharding: core = 4*b + g  (b = batch, g = head-group of 4 heads).
Per core, feature-on-partition layout throughout:
  qT,kT = matmuls of Wq/Wk vs xT;  v natural; S^T per head; softmax over keys
  via exp (no max-sub; scores ~ N(0,1)) with the denominator produced by a
  ones-column appended to v; normalized attT [C=256, S] per core.
  fc_out: each core computes its partial over the full sequence of its batch;
  the host sums the 4 partials per batch.

Schedule: software-pipelined units u=(pair, qc). Per block, AV+normalize of
unit i-1 overlaps the exp stream of unit i on ACT; projection work (v, qk of
tile 1) and fc_out chunks fill PE slack inside the ACT-bound stretches.
Softmax normalize = reciprocal_approx_fast (DVE) + partition_broadcast
(GpSimd) + one DVE multiply; per-qc fc_out spreads the output DMA.
"""

from collections import deque

import numpy as np

B, S, D, H, A = 2, 2048, 1024, 16, 64
GROUPS = 4              # head groups (cores per batch)
HPG = H // GROUPS       # heads per core = 4
C = HPG * A             # channels per core = 256
N_CORES = 8
SQ = S // GROUPS


def build_nc(s=S, d=D, n_cores=N_CORES):
    import concourse.bass as bass
    import concourse.mybir as mybir
    import concourse.tile as tile
    from concourse import bacc

    f32 = mybir.dt.float32
    bf16 = mybir.dt.bfloat16
    AF = mybir.ActivationFunctionType

    KD = d // 128        # d-tiles (contraction for projections) = 8
    MC = C // 128        # c-tiles per core = 2 (pairs of heads)
    NS = s // 128        # seq tiles = 16
    QC = s // 512        # q chunks = 4
    QW = 512             # q chunk width
    KT_PER_ST = 2        # k-tiles packed per st/pt tile (exp batching)
    NG = NS // KT_PER_ST # st/pt groups per (p, qc) = 8
    OW = 512             # output free-dim chunk

    nc = bacc.Bacc(
        "TRN2", target_bir_lowering=False, debug=False,
        enable_asserts=True, num_devices=n_cores,
    )

    xT_d = nc.dram_tensor("xT", [d, s], bf16, kind="ExternalInput").ap()
    wq_d = nc.dram_tensor("wq", [d, C], bf16, kind="ExternalInput").ap()
    wk_d = nc.dram_tensor("wk", [d, C], bf16, kind="ExternalInput").ap()
    wv_d = nc.dram_tensor("wv", [d, C], bf16, kind="ExternalInput").ap()
    wo_d = nc.dram_tensor("wo", [C, d], bf16, kind="ExternalInput").ap()
    bqs_d = nc.dram_tensor("bqs", [128, MC], f32, kind="ExternalInput").ap()
    bks_d = nc.dram_tensor("bks", [128, MC], f32, kind="ExternalInput").ap()
    bvb_d = nc.dram_tensor("bvb", [128, C], f32, kind="ExternalInput").ap()
    bob_d = nc.dram_tensor("bob", [128, d], f32, kind="ExternalInput").ap()
    out_d = nc.dram_tensor("out", [s, d], f32, kind="ExternalOutput").ap()

    with tile.TileContext(nc) as tc:
        with tc.tile_pool(name="const", bufs=1) as cpool, \
             tc.tile_pool(name="qkv", bufs=1) as qpool, \
             tc.tile_pool(name="wop", bufs=1) as wopool, \
             tc.tile_pool(name="xTw", bufs=1) as xpool, \
             tc.tile_pool(name="ptp", bufs=10) as ptpool, \
             tc.tile_pool(name="sml", bufs=3) as spool, \
             tc.tile_pool(name="osb", bufs=3) as opool, \
             tc.tile_pool(name="ps1", bufs=2, space="PSUM") as pp1, \
             tc.tile_pool(name="pst", bufs=1, space="PSUM") as stp, \
             tc.tile_pool(name="pav", bufs=1, space="PSUM") as avp:

            # warm the ACT exp table set while input DMAs are in flight
            warm_f = cpool.tile([1, 16], f32, name="warm_f")
            nc.vector.memset(warm_f[:], 0.0)
            warm_o = cpool.tile([1, 16], f32, name="warm_o")
            nc.scalar.activation(warm_o[:], warm_f[:], AF.Exp, scale=1.0)

            ones_f = cpool.tile([1, A], f32, name="ones_f")
            nc.vector.memset(ones_f[:], 1.0)
            ones_sb = cpool.tile([1, A], bf16, name="ones_sb")
            nc.scalar.copy(ones_sb[:], ones_f[:])

            bq_sb = cpool.tile([128, MC], f32, name="bq_sb")
            bk_sb = cpool.tile([128, MC], f32, name="bk_sb")
            bvb_sb = cpool.tile([128, C], f32, name="bvb_sb")
            bob_sb = cpool.tile([128, d], f32, name="bob_sb")

            qT_sb = [qpool.tile([128, s], bf16, name=f"qT{mt}", tag=f"qT{mt}")
                     for mt in range(MC)]
            kT_sb = [qpool.tile([128, s], bf16, name=f"kT{mt}", tag=f"kT{mt}")
                     for mt in range(MC)]
            # v, padded per head to 128 columns (ones col at A, zeros beyond)
            # so the AV lhsT is 128-wide and Fast Weight Load engages
            VP = 128
            v_sb = qpool.tile([128, NS, HPG, VP], bf16, name="v_sb", tag="v")
            nc.vector.memset(v_sb[:, :, :, A:VP], 0.0)
            vones_f = cpool.tile([128, NS * HPG], f32, name="vones_f")
            nc.vector.memset(vones_f[:], 1.0)
            nc.vector.tensor_copy(
                v_sb[:, :, :, A],
                vones_f[:].rearrange("p (t h) -> p t h", h=HPG))

            wo_sb = [wopool.tile([128, d], bf16, name=f"wo{kt}", tag=f"wo{kt}")
                     for kt in range(MC)]
            attn_sb = [qpool.tile([128, s], bf16, name=f"attn{t}", tag=f"at{t}")
                       for t in range(MC)]

            # ---------------- input loads ----------------
            # xT split into 4 column-chunk tiles per kt so loads and deps are
            # chunk-granular (first scores don't wait for the full xT)
            xT_sb = [[xpool.tile([128, QW], bf16, name=f"xT{kt}_{cc}",
                                 tag=f"x{kt}_{cc}") for cc in range(QC)]
                     for kt in range(KD)]
            w_sb = {}
            for wname in ("q", "k", "v"):
                w_sb[wname] = [
                    xpool.tile([128, C], bf16, name=f"w{wname}{kt}",
                               tag=f"w{wname}{kt}")
                    for kt in range(KD)
                ]
            ldeng = [nc.sync, nc.gpsimd]
            li = 0

            def load(dst, src_ap):
                nonlocal li
                ldeng[li % 2].dma_start(dst, src_ap)
                li += 1

            wds = {"q": wq_d, "k": wk_d, "v": wv_d}
            # chunk 0 of every xT tile first, so the first kT/qT chunk (and
            # the first scores) start ~8µs earlier
            for kt in range(KD):
                ks = slice(kt * 128, (kt + 1) * 128)
                load(xT_sb[kt][0][:], xT_d[ks, 0:QW])
                load(w_sb["k"][kt][:], wds["k"][ks, :])
            for kt in range(KD):
                ks = slice(kt * 128, (kt + 1) * 128)
                load(w_sb["q"][kt][:], wds["q"][ks, :])
                load(xT_sb[kt][1][:], xT_d[ks, QW:2 * QW])
            for kt in range(KD):
                ks = slice(kt * 128, (kt + 1) * 128)
                load(w_sb["v"][kt][:], wds["v"][ks, :])
                load(xT_sb[kt][2][:], xT_d[ks, 2 * QW:3 * QW])
                load(xT_sb[kt][3][:], xT_d[ks, 3 * QW:4 * QW])
            load(bq_sb[:], bqs_d[:, :])
            load(bk_sb[:], bks_d[:, :])
            load(bvb_sb[:], bvb_d[:, :])
            for kt in range(MC):
                load(wo_sb[kt][:], wo_d[kt * 128:(kt + 1) * 128, :])
            load(bob_sb[:], bob_d[:, :])

            # ---------------- building blocks ----------------
            def proj_qk_group(wname, mt, qc):
                # qT/kT[c, qs] = sum_d W[d, c] * xT[d, qs], bias via DVE
                dst, bias = ((qT_sb, bq_sb) if wname == "q" else
                             (kT_sb, bk_sb))
                qs = slice(qc * QW, (qc + 1) * QW)
                ps = pp1.tile([128, QW], f32, name="ps_qk", tag="ps_qk")
                for kt in range(KD):
                    nc.tensor.matmul(
                        ps[:],
                        lhsT=w_sb[wname][kt][:, mt * 128:(mt + 1) * 128],
                        rhs=xT_sb[kt][qc][:],
                        start=(kt == 0), stop=(kt == KD - 1),
                    )
                nc.vector.tensor_scalar_add(
                    dst[mt][:, qs], ps[:], bias[:, mt:mt + 1])

            def proj_v_group(st):
                # v[s_tile, c] = sum_d xT[d, s_tile] * Wv[d, c]
                psv = pp1.tile([128, C], f32, name="psv", tag="ps_qk")
                cc, co = divmod(st, 4)
                for kt in range(KD):
                    nc.tensor.matmul(
                        psv[:],
                        lhsT=xT_sb[kt][cc][:, co * 128:(co + 1) * 128],
                        rhs=w_sb["v"][kt][:],
                        start=(kt == 0), stop=(kt == KD - 1),
                    )
                nc.vector.tensor_add(
                    v_sb[:, st, :, 0:A],
                    psv[:].rearrange("p (h a) -> p h a", a=A),
                    bvb_sb[:].rearrange("p (h a) -> p h a", a=A),
                )

            def scores_group(p, qc, ng):
                # S^T for heads 2p (rows 0:64) / 2p+1 (rows 64:128); the two
                # K=64 matmuls hit disjoint PE row groups and co-run.
                # exp issued immediately after each head's scores.
                qs = slice(qc * QW, (qc + 1) * QW)
                sts = [stp.tile([128, KT_PER_ST * QW], f32,
                                name=f"st{hh}", tag=f"st{hh}")
                       for hh in range(2)]
                # strict T0/T8 alternation so adjacent matmuls always hit
                # disjoint PE row groups and co-run
                for jj in range(KT_PER_ST):
                    kt = ng * KT_PER_ST + jj
                    for hh in range(2):
                        off = hh * A
                        nc.tensor.matmul(
                            sts[hh][:, jj * QW:(jj + 1) * QW],
                            lhsT=kT_sb[p][off:off + A,
                                          kt * 128:(kt + 1) * 128],
                            rhs=qT_sb[p][off:off + A, qs],
                            start=True, stop=True,
                        )
                pts = []
                for hh in range(2):
                    pt = ptpool.tile([128, KT_PER_ST * QW], bf16,
                                     name=f"pt{hh}", tag=f"pt{hh}")
                    nc.scalar.activation(pt[:], sts[hh][:], AF.Exp,
                                         scale=0.125)
                    pts.append(pt)
                return pts

            def av_group(p, ng, avs, pts):
                heads = (2 * p, 2 * p + 1)
                for jj in range(KT_PER_ST):
                    kt = ng * KT_PER_ST + jj
                    for hh in range(2):
                        nc.tensor.matmul(
                            avs[hh][:],
                            lhsT=v_sb[:, kt, heads[hh], :],
                            rhs=pts[hh][:, jj * QW:(jj + 1) * QW],
                            start=(kt == 0), stop=(kt == NS - 1),
                        )

            def norm_unit(p, qc, avs):
                # attn[a, q] = av[a, q] * (1 / av[A, q]); recip on DVE,
                # column-broadcast on GpSimd, one DVE multiply.
                qs = slice(qc * QW, (qc + 1) * QW)
                for hh in range(2):
                    av = avs[hh]
                    off = hh * A
                    # custom-DVE recip must read SBUF (PSUM source gave
                    # garbage on HW) — copy the denominator row out first
                    den = spool.tile([1, QW], f32, name="den", tag="den")
                    nc.vector.tensor_copy(den[:], av[A:A + 1, :])
                    rec = spool.tile([1, QW], f32, name="rec", tag="rec")
                    nc.vector.reciprocal_approx_fast(rec[:], den[:])
                    bcd = spool.tile([A, QW], f32, name="bcd", tag="bcd")
                    nc.gpsimd.partition_broadcast(bcd[:], rec[:], channels=A)
                    nc.vector.tensor_mul(
                        attn_sb[p][off:off + A, qs], av[0:A, :], bcd[:])

            def fc_group(mt):
                # out rows [mt*128, (mt+1)*128) = attT^T @ Wo + bo
                ob = opool.tile([128, d], f32, name="ob", tag="ob")
                for nn in range(d // OW):
                    ns_ = slice(nn * OW, (nn + 1) * OW)
                    ps = pp1.tile([128, OW], f32, name="ps_o", tag="ps_qk")
                    for kt in range(MC):
                        nc.tensor.matmul(
                            ps[:],
                            lhsT=attn_sb[kt][:, mt * 128:(mt + 1) * 128],
                            rhs=wo_sb[kt][:, ns_],
                            start=(kt == 0), stop=(kt == MC - 1),
                        )
                    nc.vector.tensor_add(ob[:, ns_], ps[:], bob_sb[:, ns_])
                nc.sync.dma_start(out_d[mt * 128:(mt + 1) * 128, :], ob[:])

            # ---------------- pipelined schedule ----------------
            # filler: PE work drained into slack inside ACT-bound stretches
            filler = deque()
            done = set()

            def push(key, fn):
                filler.append((key, fn))

            def drain(n=1):
                for _ in range(n):
                    if not filler:
                        return
                    key, fn = filler.popleft()
                    fn()
                    done.add(key)

            def drain_until(key):
                while key not in done and filler:
                    k, fn = filler.popleft()
                    fn()
                    done.add(k)

            for st in range(4):
                push(("v", st), (lambda st=st: proj_v_group(st)))
            push(("qT", 0, 1), (lambda: proj_qk_group("q", 0, 1)))
            for st in range(4, 8):
                push(("v", st), (lambda st=st: proj_v_group(st)))
            push(("qT", 0, 2), (lambda: proj_qk_group("q", 0, 2)))
            for st in range(8, 12):
                push(("v", st), (lambda st=st: proj_v_group(st)))
            push(("qT", 0, 3), (lambda: proj_qk_group("q", 0, 3)))
            for st in range(12, 16):
                push(("v", st), (lambda st=st: proj_v_group(st)))
            for qc in range(QC):
                push(("kT", 1, qc), (lambda qc=qc: proj_qk_group("k", 1, qc)))
            for qc in range(QC):
                push(("qT", 1, qc), (lambda qc=qc: proj_qk_group("q", 1, qc)))

            units = [(0, 0), (0, 1), (0, 2), (1, 0), (0, 3), (1, 1),
                     (1, 2), (1, 3)]
            # fc chunk qc becomes ready once (1, qc) is normalized
            fc_ready_after = {(1, qc): qc for qc in range(QC)}

            # prologue: kT(0) chunk 0, qT(0) chunk 0
            proj_qk_group("k", 0, 0)
            done.add(("kT", 0, 0))
            proj_qk_group("q", 0, 0)
            done.add(("qT", 0, 0))

            prev = None          # (p, qc, avs, pts_list)
            for i, (p, qc) in enumerate(units):
                # AV + normalize of previous unit (overlaps this unit's exps)
                if prev is not None:
                    pp_, pqc_, pavs_, ppts_ = prev
                    for ng in range(NG):
                        for st in range(2 * ng + 1, -1, -1):
                            if ("v", st) not in done:
                                drain_until(("v", st))
                                break
                        av_group(pp_, ng, pavs_, ppts_[ng])
                        drain(1)
                    norm_unit(pp_, pqc_, pavs_)
                    if (pp_, pqc_) in fc_ready_after:
                        fqc = fc_ready_after[(pp_, pqc_)]
                        for mt in range(4 * fqc, 4 * fqc + 4):
                            filler.appendleft(
                                (("fc", mt), (lambda mt=mt: fc_group(mt))))

                # scores + exp stream of this unit
                if p == 1:
                    drain_until(("kT", 1, QC - 1))
                    drain_until(("qT", 1, qc))
                else:
                    drain_until(("qT", 0, qc))
                avs = [avp.tile([128, QW], f32, name=f"av{hh}",
                                tag=f"av{hh}") for hh in range(2)]
                pts_list = []
                for ng in range(NG):
                    need_kt = ("kT", p, min(QC - 1, (ng * KT_PER_ST + 1) // 4))
                    if p == 0 and need_kt not in done and i == 0:
                        # prologue path: issue the kT chunk directly
                        proj_qk_group("k", 0, need_kt[2])
                        done.add(need_kt)
                    pts_list.append(scores_group(p, qc, ng))
                    drain(1)
                prev = (p, qc, avs, pts_list)

            # tail: AV + normalize + fc of the last unit
            pp_, pqc_, pavs_, ppts_ = prev
            for ng in range(NG):
                av_group(pp_, ng, pavs_, ppts_[ng])
            norm_unit(pp_, pqc_, pavs_)
            while filler:
                drain(1)
            for mt in range(4 * pqc_, 4 * pqc_ + 4):
                fc_group(mt)

    nc.compile()
    return nc


def make_in_maps(x, Wq, bq, Wk, bk, Wv, bv, Wo, bo, n_cores=N_CORES):
    import ml_dtypes
    cf = ml_dtypes.bfloat16
    d = x.shape[2]
    MC = C // 128
    f = np.float32
    in_maps = []
    for core in range(n_cores):
        b, g = divmod(core, GROUPS)
        cs = slice(g * C, (g + 1) * C)
        bob = np.broadcast_to(bo, (128, d)).astype(f) if g == 0 else \
            np.zeros((128, d), f)
        m = {
            "xT": np.ascontiguousarray(x[b].T.astype(cf)),
            "wq": np.ascontiguousarray(Wq[:, cs].astype(cf)),
            "wk": np.ascontiguousarray(Wk[:, cs].astype(cf)),
            "wv": np.ascontiguousarray(Wv[:, cs].astype(cf)),
            "wo": np.ascontiguousarray(Wo[cs].astype(cf)),
            "bqs": np.ascontiguousarray(bq[cs].reshape(MC, 128).T, dtype=f),
            "bks": np.ascontiguousarray(bk[cs].reshape(MC, 128).T, dtype=f),
            "bvb": np.ascontiguousarray(np.broadcast_to(bv[cs], (128, C)), dtype=f),
            "bob": np.ascontiguousarray(bob),
        }
        in_maps.append(m)
    return in_maps


_nc_cache = {}


def _get_nc():
    if "nc" not in _nc_cache:
        _nc_cache["nc"] = build_nc()
    return _nc_cache["nc"]


def assemble(results):
    out = np.empty((B, S, D), np.float32)
    for b in range(B):
        acc = results[b * GROUPS]["out"].copy()
        for g in range(1, GROUPS):
            acc += results[b * GROUPS + g]["out"]
        out[b] = acc
    return out


def kernel(x, Wq, bq, Wk, bk, Wv, bv, Wo, bo, _trace=False, _mode=None):
    from concourse.bass_utils import run_bass_kernel_spmd

    nc = _get_nc()
    in_maps = make_in_maps(x, Wq, bq, Wk, bk, Wv, bv, Wo, bo)
    res = run_bass_kernel_spmd(nc, in_maps, core_ids=list(range(N_CORES)),
                               trace=_trace)
    _nc_cache["last_result"] = res
    return assemble(res.results)


# revision 22
# speedup vs baseline: 1.3109x; 1.0124x over previous
"""Multi-head attention (B=2,S=2048,D=1024,H=16,A=64) on 8 trn2 NeuronCores.

Sharding: core = 4*b + g  (b = batch, g = head-group of 4 heads).
Per core, feature-on-partition layout throughout:
  qT,kT = matmuls of Wq/Wk vs xT;  v natural; S^T per head; softmax over keys
  via exp (no max-sub; scores ~ N(0,1)) with the denominator produced by a
  ones-column appended to v; normalized attT [C=256, S] per core.
  fc_out: each core computes its partial over the full sequence of its batch;
  the host sums the 4 partials per batch.

Schedule: software-pipelined units u=(pair, qc). Per block, AV+normalize of
unit i-1 overlaps the exp stream of unit i on ACT; projection work (v, qk of
tile 1) and fc_out chunks fill PE slack inside the ACT-bound stretches.
Softmax normalize = reciprocal_approx_fast (DVE) + partition_broadcast
(GpSimd) + one DVE multiply; per-qc fc_out spreads the output DMA.
"""

from collections import deque

import numpy as np

B, S, D, H, A = 2, 2048, 1024, 16, 64
GROUPS = 4              # head groups (cores per batch)
HPG = H // GROUPS       # heads per core = 4
C = HPG * A             # channels per core = 256
N_CORES = 8
SQ = S // GROUPS


def build_nc(s=S, d=D, n_cores=N_CORES):
    import concourse.bass as bass
    import concourse.mybir as mybir
    import concourse.tile as tile
    from concourse import bacc

    f32 = mybir.dt.float32
    bf16 = mybir.dt.bfloat16
    AF = mybir.ActivationFunctionType

    KD = d // 128        # d-tiles (contraction for projections) = 8
    MC = C // 128        # c-tiles per core = 2 (pairs of heads)
    NS = s // 128        # seq tiles = 16
    QC = s // 512        # q chunks = 4
    QW = 512             # q chunk width
    KT_PER_ST = 2        # k-tiles packed per st/pt tile (exp batching)
    NG = NS // KT_PER_ST # st/pt groups per (p, qc) = 8
    OW = 512             # output free-dim chunk

    nc = bacc.Bacc(
        "TRN2", target_bir_lowering=False, debug=False,
        enable_asserts=True, num_devices=n_cores,
    )

    xT_d = nc.dram_tensor("xT", [d, s], bf16, kind="ExternalInput").ap()
    wq_d = nc.dram_tensor("wq", [d, C], bf16, kind="ExternalInput").ap()
    wk_d = nc.dram_tensor("wk", [d, C], bf16, kind="ExternalInput").ap()
    wv_d = nc.dram_tensor("wv", [d, C], bf16, kind="ExternalInput").ap()
    wo_d = nc.dram_tensor("wo", [C, d], bf16, kind="ExternalInput").ap()
    bqs_d = nc.dram_tensor("bqs", [128, MC], f32, kind="ExternalInput").ap()
    bks_d = nc.dram_tensor("bks", [128, MC], f32, kind="ExternalInput").ap()
    bvb_d = nc.dram_tensor("bvb", [128, C], f32, kind="ExternalInput").ap()
    bob_d = nc.dram_tensor("bob", [128, d], f32, kind="ExternalInput").ap()
    out_d = nc.dram_tensor("out", [s, d], f32, kind="ExternalOutput").ap()

    with tile.TileContext(nc) as tc:
        with tc.tile_pool(name="const", bufs=1) as cpool, \
             tc.tile_pool(name="qkv", bufs=1) as qpool, \
             tc.tile_pool(name="wop", bufs=1) as wopool, \
             tc.tile_pool(name="xTw", bufs=1) as xpool, \
             tc.tile_pool(name="ptp", bufs=10) as ptpool, \
             tc.tile_pool(name="sml", bufs=3) as spool, \
             tc.tile_pool(name="osb", bufs=3) as opool, \
             tc.tile_pool(name="ps1", bufs=2, space="PSUM") as pp1, \
             tc.tile_pool(name="pst", bufs=1, space="PSUM") as stp, \
             tc.tile_pool(name="pav", bufs=1, space="PSUM") as avp:

            # warm the ACT exp table set while input DMAs are in flight
            warm_f = cpool.tile([1, 16], f32, name="warm_f")
            nc.vector.memset(warm_f[:], 0.0)
            warm_o = cpool.tile([1, 16], f32, name="warm_o")
            nc.scalar.activation(warm_o[:], warm_f[:], AF.Exp, scale=1.0)

            ones_f = cpool.tile([1, A], f32, name="ones_f")
            nc.vector.memset(ones_f[:], 1.0)
            ones_sb = cpool.tile([1, A], bf16, name="ones_sb")
            nc.scalar.copy(ones_sb[:], ones_f[:])

            bq_sb = cpool.tile([128, MC], f32, name="bq_sb")
            bk_sb = cpool.tile([128, MC], f32, name="bk_sb")
            bvb_sb = cpool.tile([128, C], f32, name="bvb_sb")
            bob_sb = cpool.tile([128, d], f32, name="bob_sb")

            qT_sb = [qpool.tile([128, s], bf16, name=f"qT{mt}", tag=f"qT{mt}")
                     for mt in range(MC)]
            kT_sb = [qpool.tile([128, s], bf16, name=f"kT{mt}", tag=f"kT{mt}")
                     for mt in range(MC)]
            # v, padded per head to 128 columns (ones col at A, zeros beyond)
            # so the AV lhsT is 128-wide and Fast Weight Load engages
            VP = 128
            v_sb = qpool.tile([128, NS, HPG, VP], bf16, name="v_sb", tag="v")
            nc.vector.memset(v_sb[:, :, :, A:VP], 0.0)
            vones_f = cpool.tile([128, NS * HPG], f32, name="vones_f")
            nc.vector.memset(vones_f[:], 1.0)
            nc.vector.tensor_copy(
                v_sb[:, :, :, A],
                vones_f[:].rearrange("p (t h) -> p t h", h=HPG))

            wo_sb = [wopool.tile([128, d], bf16, name=f"wo{kt}", tag=f"wo{kt}")
                     for kt in range(MC)]
            attn_sb = [qpool.tile([128, s], bf16, name=f"attn{t}", tag=f"at{t}")
                       for t in range(MC)]

            # ---------------- input loads ----------------
            # xT split into 4 column-chunk tiles per kt so loads and deps are
            # chunk-granular (first scores don't wait for the full xT)
            xT_sb = [[xpool.tile([128, QW], bf16, name=f"xT{kt}_{cc}",
                                 tag=f"x{kt}_{cc}") for cc in range(QC)]
                     for kt in range(KD)]
            w_sb = {}
            for wname in ("q", "k", "v"):
                w_sb[wname] = [
                    xpool.tile([128, C], bf16, name=f"w{wname}{kt}",
                               tag=f"w{wname}{kt}")
                    for kt in range(KD)
                ]
            # HWDGE engines only: gpsimd SWDGE descriptor gen costs ~630ns
            # per load and throttles the input trickle; ACT is idle during
            # the load phase so using it is free
            ldeng = [nc.sync, nc.scalar]
            li = 0

            def load(dst, src_ap):
                nonlocal li
                ldeng[li % 2].dma_start(dst, src_ap)
                li += 1

            wds = {"q": wq_d, "k": wk_d, "v": wv_d}
            # chunk 0 of every xT tile first, so the first kT/qT chunk (and
            # the first scores) start ~8µs earlier
            for kt in range(KD):
                ks = slice(kt * 128, (kt + 1) * 128)
                load(xT_sb[kt][0][:], xT_d[ks, 0:QW])
                load(w_sb["k"][kt][:], wds["k"][ks, :])
                load(w_sb["v"][kt][:], wds["v"][ks, :])
            for kt in range(KD):
                ks = slice(kt * 128, (kt + 1) * 128)
                load(w_sb["q"][kt][:], wds["q"][ks, :])
                load(xT_sb[kt][1][:], xT_d[ks, QW:2 * QW])
            for kt in range(KD):
                ks = slice(kt * 128, (kt + 1) * 128)
                load(xT_sb[kt][2][:], xT_d[ks, 2 * QW:3 * QW])
                load(xT_sb[kt][3][:], xT_d[ks, 3 * QW:4 * QW])
            load(bq_sb[:], bqs_d[:, :])
            load(bk_sb[:], bks_d[:, :])
            load(bvb_sb[:], bvb_d[:, :])
            for kt in range(MC):
                load(wo_sb[kt][:], wo_d[kt * 128:(kt + 1) * 128, :])
            load(bob_sb[:], bob_d[:, :])

            # ---------------- building blocks ----------------
            def proj_qk_group(wname, mt, qc):
                # qT/kT[c, qs] = sum_d W[d, c] * xT[d, qs], bias via DVE
                dst, bias = ((qT_sb, bq_sb) if wname == "q" else
                             (kT_sb, bk_sb))
                qs = slice(qc * QW, (qc + 1) * QW)
                ps = pp1.tile([128, QW], f32, name="ps_qk", tag="ps_qk")
                for kt in range(KD):
                    nc.tensor.matmul(
                        ps[:],
                        lhsT=w_sb[wname][kt][:, mt * 128:(mt + 1) * 128],
                        rhs=xT_sb[kt][qc][:],
                        start=(kt == 0), stop=(kt == KD - 1),
                    )
                nc.vector.tensor_scalar_add(
                    dst[mt][:, qs], ps[:], bias[:, mt:mt + 1])

            def proj_v_group(st):
                # v[s_tile, c] = sum_d xT[d, s_tile] * Wv[d, c]
                psv = pp1.tile([128, C], f32, name="psv", tag="ps_qk")
                cc, co = divmod(st, 4)
                for kt in range(KD):
                    nc.tensor.matmul(
                        psv[:],
                        lhsT=xT_sb[kt][cc][:, co * 128:(co + 1) * 128],
                        rhs=w_sb["v"][kt][:],
                        start=(kt == 0), stop=(kt == KD - 1),
                    )
                nc.vector.tensor_add(
                    v_sb[:, st, :, 0:A],
                    psv[:].rearrange("p (h a) -> p h a", a=A),
                    bvb_sb[:].rearrange("p (h a) -> p h a", a=A),
                )

            def scores_group(p, qc, ng):
                # S^T for heads 2p (rows 0:64) / 2p+1 (rows 64:128); the two
                # K=64 matmuls hit disjoint PE row groups and co-run.
                # exp issued immediately after each head's scores.
                qs = slice(qc * QW, (qc + 1) * QW)
                sts = [stp.tile([128, KT_PER_ST * QW], f32,
                                name=f"st{hh}", tag=f"st{hh}")
                       for hh in range(2)]
                # strict T0/T8 alternation so adjacent matmuls always hit
                # disjoint PE row groups and co-run
                for jj in range(KT_PER_ST):
                    kt = ng * KT_PER_ST + jj
                    for hh in range(2):
                        off = hh * A
                        nc.tensor.matmul(
                            sts[hh][:, jj * QW:(jj + 1) * QW],
                            lhsT=kT_sb[p][off:off + A,
                                          kt * 128:(kt + 1) * 128],
                            rhs=qT_sb[p][off:off + A, qs],
                            start=True, stop=True,
                        )
                pts = []
                for hh in range(2):
                    pt = ptpool.tile([128, KT_PER_ST * QW], bf16,
                                     name=f"pt{hh}", tag=f"pt{hh}")
                    nc.scalar.activation(pt[:], sts[hh][:], AF.Exp,
                                         scale=0.125)
                    pts.append(pt)
                return pts

            def av_group(p, ng, avs, pts):
                heads = (2 * p, 2 * p + 1)
                for jj in range(KT_PER_ST):
                    kt = ng * KT_PER_ST + jj
                    for hh in range(2):
                        nc.tensor.matmul(
                            avs[hh][:],
                            lhsT=v_sb[:, kt, heads[hh], :],
                            rhs=pts[hh][:, jj * QW:(jj + 1) * QW],
                            start=(kt == 0), stop=(kt == NS - 1),
                        )

            def norm_unit(p, qc, avs):
                # attn[a, q] = av[a, q] * (1 / av[A, q]); recip on DVE,
                # column-broadcast on GpSimd, one DVE multiply.
                qs = slice(qc * QW, (qc + 1) * QW)
                for hh in range(2):
                    av = avs[hh]
                    off = hh * A
                    # custom-DVE recip must read SBUF (PSUM source gave
                    # garbage on HW) — copy the denominator row out first
                    den = spool.tile([1, QW], f32, name="den", tag="den")
                    nc.vector.tensor_copy(den[:], av[A:A + 1, :])
                    rec = spool.tile([1, QW], f32, name="rec", tag="rec")
                    nc.vector.reciprocal_approx_fast(rec[:], den[:])
                    bcd = spool.tile([A, QW], f32, name="bcd", tag="bcd")
                    nc.gpsimd.partition_broadcast(bcd[:], rec[:], channels=A)
                    nc.vector.tensor_mul(
                        attn_sb[p][off:off + A, qs], av[0:A, :], bcd[:])

            def fc_group(mt):
                # out rows [mt*128, (mt+1)*128) = attT^T @ Wo + bo
                ob = opool.tile([128, d], f32, name="ob", tag="ob")
                for nn in range(d // OW):
                    ns_ = slice(nn * OW, (nn + 1) * OW)
                    ps = pp1.tile([128, OW], f32, name="ps_o", tag="ps_qk")
                    for kt in range(MC):
                        nc.tensor.matmul(
                            ps[:],
                            lhsT=attn_sb[kt][:, mt * 128:(mt + 1) * 128],
                            rhs=wo_sb[kt][:, ns_],
                            start=(kt == 0), stop=(kt == MC - 1),
                        )
                    nc.vector.tensor_add(ob[:, ns_], ps[:], bob_sb[:, ns_])
                nc.sync.dma_start(out_d[mt * 128:(mt + 1) * 128, :], ob[:])

            # ---------------- pipelined schedule ----------------
            # filler: PE work drained into slack inside ACT-bound stretches
            filler = deque()
            done = set()

            def push(key, fn):
                filler.append((key, fn))

            def drain(n=1):
                for _ in range(n):
                    if not filler:
                        return
                    key, fn = filler.popleft()
                    fn()
                    done.add(key)

            def drain_until(key):
                while key not in done and filler:
                    k, fn = filler.popleft()
                    fn()
                    done.add(k)

            for st in range(4, 16):
                push(("v", st), (lambda st=st: proj_v_group(st)))
            for qc in range(1, QC):
                push(("qT", 0, qc), (lambda qc=qc: proj_qk_group("q", 0, qc)))
            for qc in range(QC):
                push(("kT", 1, qc), (lambda qc=qc: proj_qk_group("k", 1, qc)))
            for qc in range(QC):
                push(("qT", 1, qc), (lambda qc=qc: proj_qk_group("q", 1, qc)))

            units = [(0, 0), (0, 1), (0, 2), (1, 0), (0, 3), (1, 1),
                     (1, 2), (1, 3)]
            # fc chunk qc becomes ready once (1, qc) is normalized
            fc_ready_after = {(1, qc): qc for qc in range(QC)}

            # prologue: kT(0) chunk 0, first v chunks, qT(0) chunk 0
            proj_qk_group("k", 0, 0)
            done.add(("kT", 0, 0))
            for st in range(4):
                proj_v_group(st)
                done.add(("v", st))
            proj_qk_group("q", 0, 0)
            done.add(("qT", 0, 0))

            for i, (p, qc) in enumerate(units):
                # prereqs of this unit's scores
                if p == 1:
                    drain_until(("kT", 1, QC - 1))
                    drain_until(("qT", 1, qc))
                else:
                    drain_until(("qT", 0, qc))
                avs = [avp.tile([128, QW], f32, name=f"av{hh}",
                                tag=f"av{hh}") for hh in range(2)]
                pts_list = []
                # AV lags its own scores by one ng: av(ng-1) runs right as
                # exp(ng-1) completes, so ACT never waits across phases
                for ng in range(NG):
                    need_kt = ("kT", p, min(QC - 1, (ng * KT_PER_ST + 1) // 4))
                    if p == 0 and need_kt not in done and i == 0:
                        proj_qk_group("k", 0, need_kt[2])
                        done.add(need_kt)
                    pts_list.append(scores_group(p, qc, ng))
                    if i == 0:
                        drain_until(("v", 2 * ng + 1))
                    else:
                        drain(1)
                    if ng > 0:
                        av_group(p, ng - 1, avs, pts_list[ng - 1])
                av_group(p, NG - 1, avs, pts_list[NG - 1])
                norm_unit(p, qc, avs)
                if (p, qc) in fc_ready_after:
                    fqc = fc_ready_after[(p, qc)]
                    for mt in range(4 * fqc, 4 * fqc + 4):
                        filler.appendleft(
                            (("fc", mt), (lambda mt=mt: fc_group(mt))))

            # tail: leftover filler (last fc chunk)
            while filler:
                drain(1)

    nc.compile()
    return nc


def make_in_maps(x, Wq, bq, Wk, bk, Wv, bv, Wo, bo, n_cores=N_CORES):
    import ml_dtypes
    cf = ml_dtypes.bfloat16
    d = x.shape[2]
    MC = C // 128
    f = np.float32
    in_maps = []
    for core in range(n_cores):
        b, g = divmod(core, GROUPS)
        cs = slice(g * C, (g + 1) * C)
        bob = np.broadcast_to(bo, (128, d)).astype(f) if g == 0 else \
            np.zeros((128, d), f)
        m = {
            "xT": np.ascontiguousarray(x[b].T.astype(cf)),
            "wq": np.ascontiguousarray(Wq[:, cs].astype(cf)),
            "wk": np.ascontiguousarray(Wk[:, cs].astype(cf)),
            "wv": np.ascontiguousarray(Wv[:, cs].astype(cf)),
            "wo": np.ascontiguousarray(Wo[cs].astype(cf)),
            "bqs": np.ascontiguousarray(bq[cs].reshape(MC, 128).T, dtype=f),
            "bks": np.ascontiguousarray(bk[cs].reshape(MC, 128).T, dtype=f),
            "bvb": np.ascontiguousarray(np.broadcast_to(bv[cs], (128, C)), dtype=f),
            "bob": np.ascontiguousarray(bob),
        }
        in_maps.append(m)
    return in_maps


_nc_cache = {}


def _get_nc():
    if "nc" not in _nc_cache:
        _nc_cache["nc"] = build_nc()
    return _nc_cache["nc"]


def assemble(results):
    out = np.empty((B, S, D), np.float32)
    for b in range(B):
        acc = results[b * GROUPS]["out"].copy()
        for g in range(1, GROUPS):
            acc += results[b * GROUPS + g]["out"]
        out[b] = acc
    return out


def kernel(x, Wq, bq, Wk, bk, Wv, bv, Wo, bo, _trace=False, _mode=None):
    from concourse.bass_utils import run_bass_kernel_spmd

    nc = _get_nc()
    in_maps = make_in_maps(x, Wq, bq, Wk, bk, Wv, bv, Wo, bo)
    res = run_bass_kernel_spmd(nc, in_maps, core_ids=list(range(N_CORES)),
                               trace=_trace)
    _nc_cache["last_result"] = res
    return assemble(res.results)


# revision 25
# speedup vs baseline: 1.3469x; 1.0275x over previous
"""Multi-head attention (B=2,S=2048,D=1024,H=16,A=64) on 8 trn2 NeuronCores.

Sharding: core = 4*b + g  (b = batch, g = head-group of 4 heads).
Per core, feature-on-partition layout throughout:
  qT,kT = matmuls of Wq/Wk vs xT;  v natural; S^T per head; softmax over keys
  via exp (no max-sub; scores ~ N(0,1)) with the denominator produced by a
  ones-column appended to v; normalized attT [C=256, S] per core.
  fc_out: each core computes its partial over the full sequence of its batch;
  the host sums the 4 partials per batch.

Schedule: software-pipelined units u=(pair, qc). Per block, AV+normalize of
unit i-1 overlaps the exp stream of unit i on ACT; projection work (v, qk of
tile 1) and fc_out chunks fill PE slack inside the ACT-bound stretches.
Softmax normalize = reciprocal_approx_fast (DVE) + partition_broadcast
(GpSimd) + one DVE multiply; per-qc fc_out spreads the output DMA.
"""

from collections import deque

import numpy as np

B, S, D, H, A = 2, 2048, 1024, 16, 64
GROUPS = 4              # head groups (cores per batch)
HPG = H // GROUPS       # heads per core = 4
C = HPG * A             # channels per core = 256
N_CORES = 8
SQ = S // GROUPS


def build_nc(s=S, d=D, n_cores=N_CORES):
    import concourse.bass as bass
    import concourse.mybir as mybir
    import concourse.tile as tile
    from concourse import bacc

    f32 = mybir.dt.float32
    bf16 = mybir.dt.bfloat16
    AF = mybir.ActivationFunctionType

    KD = d // 128        # d-tiles (contraction for projections) = 8
    MC = C // 128        # c-tiles per core = 2 (pairs of heads)
    NS = s // 128        # seq tiles = 16
    QC = s // 512        # q chunks = 4
    QW = 512             # q chunk width
    KT_PER_ST = 2        # k-tiles packed per st/pt tile (exp batching)
    NG = NS // KT_PER_ST # st/pt groups per (p, qc) = 8
    OW = 512             # output free-dim chunk

    nc = bacc.Bacc(
        "TRN2", target_bir_lowering=False, debug=False,
        enable_asserts=True, num_devices=n_cores,
    )

    xT_d = nc.dram_tensor("xT", [d, s], bf16, kind="ExternalInput").ap()
    wq_d = nc.dram_tensor("wq", [d, C], bf16, kind="ExternalInput").ap()
    wk_d = nc.dram_tensor("wk", [d, C], bf16, kind="ExternalInput").ap()
    wv_d = nc.dram_tensor("wv", [d, C], bf16, kind="ExternalInput").ap()
    wo_d = nc.dram_tensor("wo", [C, d], bf16, kind="ExternalInput").ap()
    bqs_d = nc.dram_tensor("bqs", [128, MC], f32, kind="ExternalInput").ap()
    bks_d = nc.dram_tensor("bks", [128, MC], f32, kind="ExternalInput").ap()
    bvb_d = nc.dram_tensor("bvb", [128, C], f32, kind="ExternalInput").ap()
    bob_d = nc.dram_tensor("bob", [128, d], f32, kind="ExternalInput").ap()
    out_d = nc.dram_tensor("out", [s, d], f32, kind="ExternalOutput").ap()

    with tile.TileContext(nc) as tc:
        with tc.tile_pool(name="const", bufs=1) as cpool, \
             tc.tile_pool(name="qkv", bufs=1) as qpool, \
             tc.tile_pool(name="wop", bufs=1) as wopool, \
             tc.tile_pool(name="xTw", bufs=1) as xpool, \
             tc.tile_pool(name="ptp", bufs=10) as ptpool, \
             tc.tile_pool(name="sml", bufs=3) as spool, \
             tc.tile_pool(name="osb", bufs=3) as opool, \
             tc.tile_pool(name="ps1", bufs=2, space="PSUM") as pp1, \
             tc.tile_pool(name="pst", bufs=1, space="PSUM") as stp, \
             tc.tile_pool(name="pav", bufs=1, space="PSUM") as avp:

            # warm the ACT exp table set while input DMAs are in flight
            warm_f = cpool.tile([1, 16], f32, name="warm_f")
            nc.vector.memset(warm_f[:], 0.0)
            warm_o = cpool.tile([1, 16], f32, name="warm_o")
            nc.scalar.activation(warm_o[:], warm_f[:], AF.Exp, scale=1.0)

            ones_f = cpool.tile([1, A], f32, name="ones_f")
            nc.vector.memset(ones_f[:], 1.0)
            ones_sb = cpool.tile([1, A], bf16, name="ones_sb")
            nc.scalar.copy(ones_sb[:], ones_f[:])

            bq_sb = cpool.tile([128, MC], f32, name="bq_sb")
            bk_sb = cpool.tile([128, MC], f32, name="bk_sb")
            bvb_sb = cpool.tile([128, C], f32, name="bvb_sb")
            bob_sb = cpool.tile([128, d], f32, name="bob_sb")

            qT_sb = [qpool.tile([128, s], bf16, name=f"qT{mt}", tag=f"qT{mt}")
                     for mt in range(MC)]
            kT_sb = [qpool.tile([128, s], bf16, name=f"kT{mt}", tag=f"kT{mt}")
                     for mt in range(MC)]
            # v, padded per head to 128 columns (ones col at A, zeros beyond)
            # so the AV lhsT is 128-wide and Fast Weight Load engages
            VP = 128
            v_sb = qpool.tile([128, NS, HPG, VP], bf16, name="v_sb", tag="v")
            nc.vector.memset(v_sb[:, :, :, A:VP], 0.0)
            vones_f = cpool.tile([128, NS * HPG], f32, name="vones_f")
            nc.vector.memset(vones_f[:], 1.0)
            nc.vector.tensor_copy(
                v_sb[:, :, :, A],
                vones_f[:].rearrange("p (t h) -> p t h", h=HPG))

            wo_sb = [wopool.tile([128, d], bf16, name=f"wo{kt}", tag=f"wo{kt}")
                     for kt in range(MC)]
            attn_sb = [qpool.tile([128, s], bf16, name=f"attn{t}", tag=f"at{t}")
                       for t in range(MC)]

            # ---------------- input loads ----------------
            # xT split into 4 column-chunk tiles per kt so loads and deps are
            # chunk-granular (first scores don't wait for the full xT)
            xT_sb = [[xpool.tile([128, QW], bf16, name=f"xT{kt}_{cc}",
                                 tag=f"x{kt}_{cc}") for cc in range(QC)]
                     for kt in range(KD)]
            w_sb = {}
            for wname in ("q", "k", "v"):
                w_sb[wname] = [
                    xpool.tile([128, C], bf16, name=f"w{wname}{kt}",
                               tag=f"w{wname}{kt}")
                    for kt in range(KD)
                ]
            # HWDGE engines only: gpsimd SWDGE descriptor gen costs ~630ns
            # per load and throttles the input trickle; ACT is idle during
            # the load phase so using it is free
            ldeng = [nc.sync, nc.scalar]
            li = 0

            def load(dst, src_ap):
                nonlocal li
                ldeng[li % 2].dma_start(dst, src_ap)
                li += 1

            wds = {"q": wq_d, "k": wk_d, "v": wv_d}
            # chunk 0 of every xT tile first, so the first kT/qT chunk (and
            # the first scores) start ~8µs earlier
            for kt in range(KD):
                ks = slice(kt * 128, (kt + 1) * 128)
                load(xT_sb[kt][0][:], xT_d[ks, 0:QW])
                load(w_sb["k"][kt][:], wds["k"][ks, :])
            for kt in range(KD):
                ks = slice(kt * 128, (kt + 1) * 128)
                load(w_sb["q"][kt][:], wds["q"][ks, :])
                load(w_sb["v"][kt][:], wds["v"][ks, :])
            for kt in range(KD):
                ks = slice(kt * 128, (kt + 1) * 128)
                load(xT_sb[kt][1][:], xT_d[ks, QW:2 * QW])
                load(xT_sb[kt][2][:], xT_d[ks, 2 * QW:3 * QW])
                load(xT_sb[kt][3][:], xT_d[ks, 3 * QW:4 * QW])
            load(bq_sb[:], bqs_d[:, :])
            load(bk_sb[:], bks_d[:, :])
            load(bvb_sb[:], bvb_d[:, :])
            for kt in range(MC):
                load(wo_sb[kt][:], wo_d[kt * 128:(kt + 1) * 128, :])
            load(bob_sb[:], bob_d[:, :])

            # ---------------- building blocks ----------------
            def proj_qk_group(wname, mt, qc):
                # qT/kT[c, qs] = sum_d W[d, c] * xT[d, qs], bias via DVE
                dst, bias = ((qT_sb, bq_sb) if wname == "q" else
                             (kT_sb, bk_sb))
                qs = slice(qc * QW, (qc + 1) * QW)
                ps = pp1.tile([128, QW], f32, name="ps_qk", tag="ps_qk")
                for kt in range(KD):
                    nc.tensor.matmul(
                        ps[:],
                        lhsT=w_sb[wname][kt][:, mt * 128:(mt + 1) * 128],
                        rhs=xT_sb[kt][qc][:],
                        start=(kt == 0), stop=(kt == KD - 1),
                    )
                nc.vector.tensor_scalar_add(
                    dst[mt][:, qs], ps[:], bias[:, mt:mt + 1])

            def proj_v_group(st):
                # v[s_tile, c] = sum_d xT[d, s_tile] * Wv[d, c]
                psv = pp1.tile([128, C], f32, name="psv", tag="ps_qk")
                cc, co = divmod(st, 4)
                for kt in range(KD):
                    nc.tensor.matmul(
                        psv[:],
                        lhsT=xT_sb[kt][cc][:, co * 128:(co + 1) * 128],
                        rhs=w_sb["v"][kt][:],
                        start=(kt == 0), stop=(kt == KD - 1),
                    )
                nc.vector.tensor_add(
                    v_sb[:, st, :, 0:A],
                    psv[:].rearrange("p (h a) -> p h a", a=A),
                    bvb_sb[:].rearrange("p (h a) -> p h a", a=A),
                )

            def scores_group(p, qc, ng):
                # S^T for heads 2p (rows 0:64) / 2p+1 (rows 64:128); the two
                # K=64 matmuls hit disjoint PE row groups and co-run.
                # exp issued immediately after each head's scores.
                qs = slice(qc * QW, (qc + 1) * QW)
                sts = [stp.tile([128, KT_PER_ST * QW], f32,
                                name=f"st{hh}", tag=f"st{hh}")
                       for hh in range(2)]
                # strict T0/T8 alternation so adjacent matmuls always hit
                # disjoint PE row groups and co-run
                for jj in range(KT_PER_ST):
                    kt = ng * KT_PER_ST + jj
                    for hh in range(2):
                        off = hh * A
                        nc.tensor.matmul(
                            sts[hh][:, jj * QW:(jj + 1) * QW],
                            lhsT=kT_sb[p][off:off + A,
                                          kt * 128:(kt + 1) * 128],
                            rhs=qT_sb[p][off:off + A, qs],
                            start=True, stop=True,
                        )
                pts = []
                for hh in range(2):
                    pt = ptpool.tile([128, KT_PER_ST * QW], bf16,
                                     name=f"pt{hh}", tag=f"pt{hh}")
                    nc.scalar.activation(pt[:], sts[hh][:], AF.Exp,
                                         scale=0.125)
                    pts.append(pt)
                return pts

            def av_group(p, ng, avs, pts):
                heads = (2 * p, 2 * p + 1)
                for jj in range(KT_PER_ST):
                    kt = ng * KT_PER_ST + jj
                    for hh in range(2):
                        nc.tensor.matmul(
                            avs[hh][:],
                            lhsT=v_sb[:, kt, heads[hh], :],
                            rhs=pts[hh][:, jj * QW:(jj + 1) * QW],
                            start=(kt == 0), stop=(kt == NS - 1),
                        )

            def norm_unit(p, qc, avs):
                # attn[a, q] = av[a, q] * (1 / av[A, q]); recip on DVE,
                # column-broadcast on GpSimd, one DVE multiply.
                qs = slice(qc * QW, (qc + 1) * QW)
                for hh in range(2):
                    av = avs[hh]
                    off = hh * A
                    # custom-DVE recip must read SBUF (PSUM source gave
                    # garbage on HW) — copy the denominator row out first
                    den = spool.tile([1, QW], f32, name="den", tag="den")
                    nc.vector.tensor_copy(den[:], av[A:A + 1, :])
                    rec = spool.tile([1, QW], f32, name="rec", tag="rec")
                    nc.vector.reciprocal_approx_fast(rec[:], den[:])
                    bcd = spool.tile([A, QW], f32, name="bcd", tag="bcd")
                    nc.gpsimd.partition_broadcast(bcd[:], rec[:], channels=A)
                    nc.vector.tensor_mul(
                        attn_sb[p][off:off + A, qs], av[0:A, :], bcd[:])

            def fc_group(mt):
                # out rows [mt*128, (mt+1)*128) = attT^T @ Wo + bo
                ob = opool.tile([128, d], f32, name="ob", tag="ob")
                for nn in range(d // OW):
                    ns_ = slice(nn * OW, (nn + 1) * OW)
                    ps = pp1.tile([128, OW], f32, name="ps_o", tag="ps_qk")
                    for kt in range(MC):
                        nc.tensor.matmul(
                            ps[:],
                            lhsT=attn_sb[kt][:, mt * 128:(mt + 1) * 128],
                            rhs=wo_sb[kt][:, ns_],
                            start=(kt == 0), stop=(kt == MC - 1),
                        )
                    nc.vector.tensor_add(ob[:, ns_], ps[:], bob_sb[:, ns_])
                nc.sync.dma_start(out_d[mt * 128:(mt + 1) * 128, :], ob[:])

            # ---------------- pipelined schedule ----------------
            # filler: PE work drained into slack inside ACT-bound stretches
            filler = deque()
            done = set()

            def push(key, fn):
                filler.append((key, fn))

            def drain(n=1):
                for _ in range(n):
                    if not filler:
                        return
                    key, fn = filler.popleft()
                    fn()
                    done.add(key)

            def drain_until(key):
                while key not in done and filler:
                    k, fn = filler.popleft()
                    fn()
                    done.add(k)

            for st in range(16):
                push(("v", st), (lambda st=st: proj_v_group(st)))
            for qc in range(1, QC):
                push(("qT", 0, qc), (lambda qc=qc: proj_qk_group("q", 0, qc)))
            for qc in range(QC):
                push(("kT", 1, qc), (lambda qc=qc: proj_qk_group("k", 1, qc)))
            for qc in range(QC):
                push(("qT", 1, qc), (lambda qc=qc: proj_qk_group("q", 1, qc)))

            units = [(0, 0), (0, 1), (0, 2), (1, 0), (0, 3), (1, 1),
                     (1, 2), (1, 3)]
            # fc chunk qc becomes ready once (1, qc) is normalized
            fc_ready_after = {(1, qc): qc for qc in range(QC)}

            # prologue: kT(0) chunk 0, qT(0) chunk 0
            proj_qk_group("k", 0, 0)
            done.add(("kT", 0, 0))
            proj_qk_group("q", 0, 0)
            done.add(("qT", 0, 0))

            for i, (p, qc) in enumerate(units):
                # prereqs of this unit's scores
                if p == 1:
                    drain_until(("kT", 1, QC - 1))
                    drain_until(("qT", 1, qc))
                else:
                    drain_until(("qT", 0, qc))
                avs = [avp.tile([128, QW], f32, name=f"av{hh}",
                                tag=f"av{hh}") for hh in range(2)]
                pts_list = []
                # AV lags its own scores by one ng: av(ng-1) runs right as
                # exp(ng-1) completes, so ACT never waits across phases
                for ng in range(NG):
                    need_kt = ("kT", p, min(QC - 1, (ng * KT_PER_ST + 1) // 4))
                    if p == 0 and need_kt not in done and i == 0:
                        proj_qk_group("k", 0, need_kt[2])
                        done.add(need_kt)
                    pts_list.append(scores_group(p, qc, ng))
                    if i == 0:
                        drain_until(("v", 2 * ng + 1))
                    else:
                        drain(1)
                    if ng > 0:
                        av_group(p, ng - 1, avs, pts_list[ng - 1])
                av_group(p, NG - 1, avs, pts_list[NG - 1])
                norm_unit(p, qc, avs)
                if (p, qc) in fc_ready_after:
                    fqc = fc_ready_after[(p, qc)]
                    for mt in range(4 * fqc, 4 * fqc + 4):
                        filler.appendleft(
                            (("fc", mt), (lambda mt=mt: fc_group(mt))))

            # tail: leftover filler (last fc chunk)
            while filler:
                drain(1)

    nc.compile()
    return nc


def make_in_maps(x, Wq, bq, Wk, bk, Wv, bv, Wo, bo, n_cores=N_CORES):
    import ml_dtypes
    cf = ml_dtypes.bfloat16
    d = x.shape[2]
    MC = C // 128
    f = np.float32
    in_maps = []
    for core in range(n_cores):
        b, g = divmod(core, GROUPS)
        cs = slice(g * C, (g + 1) * C)
        bob = np.broadcast_to(bo, (128, d)).astype(f) if g == 0 else \
            np.zeros((128, d), f)
        m = {
            "xT": np.ascontiguousarray(x[b].T.astype(cf)),
            "wq": np.ascontiguousarray(Wq[:, cs].astype(cf)),
            "wk": np.ascontiguousarray(Wk[:, cs].astype(cf)),
            "wv": np.ascontiguousarray(Wv[:, cs].astype(cf)),
            "wo": np.ascontiguousarray(Wo[cs].astype(cf)),
            "bqs": np.ascontiguousarray(bq[cs].reshape(MC, 128).T, dtype=f),
            "bks": np.ascontiguousarray(bk[cs].reshape(MC, 128).T, dtype=f),
            "bvb": np.ascontiguousarray(np.broadcast_to(bv[cs], (128, C)), dtype=f),
            "bob": np.ascontiguousarray(bob),
        }
        in_maps.append(m)
    return in_maps


_nc_cache = {}


def _get_nc():
    if "nc" not in _nc_cache:
        _nc_cache["nc"] = build_nc()
    return _nc_cache["nc"]


def assemble(results):
    out = np.empty((B, S, D), np.float32)
    for b in range(B):
        acc = results[b * GROUPS]["out"].copy()
        for g in range(1, GROUPS):
            acc += results[b * GROUPS + g]["out"]
        out[b] = acc
    return out


def kernel(x, Wq, bq, Wk, bk, Wv, bv, Wo, bo, _trace=False, _mode=None):
    from concourse.bass_utils import run_bass_kernel_spmd

    nc = _get_nc()
    in_maps = make_in_maps(x, Wq, bq, Wk, bk, Wv, bv, Wo, bo)
    res = run_bass_kernel_spmd(nc, in_maps, core_ids=list(range(N_CORES)),
                               trace=_trace)
    _nc_cache["last_result"] = res
    return assemble(res.results)


# revision 29
# speedup vs baseline: 1.3796x; 1.0243x over previous
"""Multi-head attention (B=2,S=2048,D=1024,H=16,A=64) on 8 trn2 NeuronCores.

Sharding: core = 4*b + g  (b = batch, g = head-group of 4 heads).
Per core, feature-on-partition layout throughout:
  qT,kT = matmuls of Wq/Wk vs xT;  v natural; S^T per head; softmax over keys
  via exp (no max-sub; scores ~ N(0,1)) with the denominator produced by a
  ones-column appended to v; normalized attT [C=256, S] per core.
  fc_out: each core computes its partial over the full sequence of its batch;
  the host sums the 4 partials per batch.

Schedule: software-pipelined units u=(pair, qc). Per block, AV+normalize of
unit i-1 overlaps the exp stream of unit i on ACT; projection work (v, qk of
tile 1) and fc_out chunks fill PE slack inside the ACT-bound stretches.
Softmax normalize = reciprocal_approx_fast (DVE) + partition_broadcast
(GpSimd) + one DVE multiply; per-qc fc_out spreads the output DMA.
"""

from collections import deque

import numpy as np

B, S, D, H, A = 2, 2048, 1024, 16, 64
GROUPS = 4              # head groups (cores per batch)
HPG = H // GROUPS       # heads per core = 4
C = HPG * A             # channels per core = 256
N_CORES = 8
SQ = S // GROUPS


def build_nc(s=S, d=D, n_cores=N_CORES):
    import concourse.bass as bass
    import concourse.mybir as mybir
    import concourse.tile as tile
    from concourse import bacc

    f32 = mybir.dt.float32
    bf16 = mybir.dt.bfloat16
    AF = mybir.ActivationFunctionType

    KD = d // 128        # d-tiles (contraction for projections) = 8
    MC = C // 128        # c-tiles per core = 2 (pairs of heads)
    NS = s // 128        # seq tiles = 16
    QC = s // 512        # q chunks = 4
    QW = 512             # q chunk width
    KT_PER_ST = 2        # k-tiles packed per st/pt tile (exp batching)
    NG = NS // KT_PER_ST # st/pt groups per (p, qc) = 8
    OW = 512             # output free-dim chunk

    nc = bacc.Bacc(
        "TRN2", target_bir_lowering=False, debug=False,
        enable_asserts=True, num_devices=n_cores,
    )

    xT_d = nc.dram_tensor("xT", [d, s], bf16, kind="ExternalInput").ap()
    wq_d = nc.dram_tensor("wq", [d, C], bf16, kind="ExternalInput").ap()
    wk_d = nc.dram_tensor("wk", [d, C], bf16, kind="ExternalInput").ap()
    wv_d = nc.dram_tensor("wv", [d, C], bf16, kind="ExternalInput").ap()
    wo_d = nc.dram_tensor("wo", [C, d], bf16, kind="ExternalInput").ap()
    bqs_d = nc.dram_tensor("bqs", [128, MC], f32, kind="ExternalInput").ap()
    bks_d = nc.dram_tensor("bks", [128, MC], f32, kind="ExternalInput").ap()
    bvb_d = nc.dram_tensor("bvb", [128, C], f32, kind="ExternalInput").ap()
    bob_d = nc.dram_tensor("bob", [128, d], f32, kind="ExternalInput").ap()
    out_d = nc.dram_tensor("out", [s, d], f32, kind="ExternalOutput").ap()

    with tile.TileContext(nc) as tc:
        with tc.tile_pool(name="const", bufs=1) as cpool, \
             tc.tile_pool(name="qkv", bufs=1) as qpool, \
             tc.tile_pool(name="wop", bufs=1) as wopool, \
             tc.tile_pool(name="xTw", bufs=1) as xpool, \
             tc.tile_pool(name="ptp", bufs=10) as ptpool, \
             tc.tile_pool(name="sml", bufs=3) as spool, \
             tc.tile_pool(name="osb", bufs=3) as opool, \
             tc.tile_pool(name="ps1", bufs=2, space="PSUM") as pp1, \
             tc.tile_pool(name="pst", bufs=1, space="PSUM") as stp, \
             tc.tile_pool(name="pav", bufs=1, space="PSUM") as avp:

            # warm the ACT exp table set while input DMAs are in flight
            warm_f = cpool.tile([1, 16], f32, name="warm_f")
            nc.vector.memset(warm_f[:], 0.0)
            warm_o = cpool.tile([1, 16], f32, name="warm_o")
            nc.scalar.activation(warm_o[:], warm_f[:], AF.Exp, scale=1.0)

            ones_f = cpool.tile([1, A], f32, name="ones_f")
            nc.vector.memset(ones_f[:], 1.0)
            ones_sb = cpool.tile([1, A], bf16, name="ones_sb")
            nc.scalar.copy(ones_sb[:], ones_f[:])

            bq_sb = cpool.tile([128, MC], f32, name="bq_sb")
            bk_sb = cpool.tile([128, MC], f32, name="bk_sb")
            bvb_sb = cpool.tile([128, C], f32, name="bvb_sb")
            bob_sb = cpool.tile([128, d], f32, name="bob_sb")

            # qT zero-padded per head-half: qTz[p][hh] holds head 2p+hh in
            # rows hh*A..hh*A+A, zeros elsewhere. Scores then contract the
            # full 128 rows of kT (zeros annihilate the other head), keeping
            # the PE in uniform 128x128 mode — no row-tiling mode switches.
            qTz_sb = [[qpool.tile([128, s], bf16, name=f"qTz{mt}_{hh}",
                                  tag=f"qTz{mt}_{hh}") for hh in range(2)]
                      for mt in range(MC)]
            for mt in range(MC):
                for hh in range(2):
                    nc.vector.memset(
                        qTz_sb[mt][hh][(1 - hh) * A:(2 - hh) * A, :], 0.0)
            kT_sb = [qpool.tile([128, s], bf16, name=f"kT{mt}", tag=f"kT{mt}")
                     for mt in range(MC)]
            # v, padded per head to 128 columns (ones col at A, zeros beyond)
            # so the AV lhsT is 128-wide and Fast Weight Load engages
            VP = 128
            v_sb = qpool.tile([128, NS, HPG, VP], bf16, name="v_sb", tag="v")
            nc.vector.memset(v_sb[:, :, :, A:VP], 0.0)
            vones_f = cpool.tile([128, NS * HPG], f32, name="vones_f")
            nc.vector.memset(vones_f[:], 1.0)
            nc.vector.tensor_copy(
                v_sb[:, :, :, A],
                vones_f[:].rearrange("p (t h) -> p t h", h=HPG))

            wo_sb = [wopool.tile([128, d], bf16, name=f"wo{kt}", tag=f"wo{kt}")
                     for kt in range(MC)]
            attn_sb = [qpool.tile([128, s], bf16, name=f"attn{t}", tag=f"at{t}")
                       for t in range(MC)]

            # ---------------- input loads ----------------
            # xT split into 4 column-chunk tiles per kt so loads and deps are
            # chunk-granular (first scores don't wait for the full xT)
            xT_sb = [[xpool.tile([128, QW], bf16, name=f"xT{kt}_{cc}",
                                 tag=f"x{kt}_{cc}") for cc in range(QC)]
                     for kt in range(KD)]
            w_sb = {}
            for wname in ("q", "k", "v"):
                w_sb[wname] = [
                    xpool.tile([128, C], bf16, name=f"w{wname}{kt}",
                               tag=f"w{wname}{kt}")
                    for kt in range(KD)
                ]
            # HWDGE engines only: gpsimd SWDGE descriptor gen costs ~630ns
            # per load and throttles the input trickle; ACT is idle during
            # the load phase so using it is free
            ldeng = [nc.sync, nc.scalar]
            li = 0

            def load(dst, src_ap):
                nonlocal li
                ldeng[li % 2].dma_start(dst, src_ap)
                li += 1

            wds = {"q": wq_d, "k": wk_d, "v": wv_d}
            # chunk 0 of every xT tile first, so the first kT/qT chunk (and
            # the first scores) start ~8µs earlier
            for kt in range(KD):
                ks = slice(kt * 128, (kt + 1) * 128)
                load(xT_sb[kt][0][:], xT_d[ks, 0:QW])
                load(w_sb["k"][kt][:], wds["k"][ks, :])
            for kt in range(KD):
                ks = slice(kt * 128, (kt + 1) * 128)
                load(w_sb["q"][kt][:], wds["q"][ks, :])
                load(w_sb["v"][kt][:], wds["v"][ks, :])
            for kt in range(KD):
                ks = slice(kt * 128, (kt + 1) * 128)
                load(xT_sb[kt][1][:], xT_d[ks, QW:2 * QW])
                load(xT_sb[kt][2][:], xT_d[ks, 2 * QW:3 * QW])
                load(xT_sb[kt][3][:], xT_d[ks, 3 * QW:4 * QW])
            load(bq_sb[:], bqs_d[:, :])
            load(bk_sb[:], bks_d[:, :])
            load(bvb_sb[:], bvb_d[:, :])
            for kt in range(MC):
                load(wo_sb[kt][:], wo_d[kt * 128:(kt + 1) * 128, :])
            load(bob_sb[:], bob_d[:, :])

            # ---------------- building blocks ----------------
            def proj_qk_group(wname, mt, qc):
                # qT/kT[c, qs] = sum_d W[d, c] * xT[d, qs], bias via DVE
                qs = slice(qc * QW, (qc + 1) * QW)
                ps = pp1.tile([128, QW], f32, name="ps_qk", tag="ps_qk")
                for kt in range(KD):
                    nc.tensor.matmul(
                        ps[:],
                        lhsT=w_sb[wname][kt][:, mt * 128:(mt + 1) * 128],
                        rhs=xT_sb[kt][qc][:],
                        start=(kt == 0), stop=(kt == KD - 1),
                    )
                if wname == "k":
                    nc.vector.tensor_scalar_add(
                        kT_sb[mt][:, qs], ps[:], bk_sb[:, mt:mt + 1])
                else:
                    for hh in range(2):
                        rs = slice(hh * A, (hh + 1) * A)
                        nc.vector.tensor_scalar_add(
                            qTz_sb[mt][hh][rs, qs], ps[rs, :],
                            bq_sb[rs, mt:mt + 1])

            def proj_v_group(st):
                # v[s_tile, c] = sum_d xT[d, s_tile] * Wv[d, c]
                psv = pp1.tile([128, C], f32, name="psv", tag="ps_qk")
                cc, co = divmod(st, 4)
                for kt in range(KD):
                    nc.tensor.matmul(
                        psv[:],
                        lhsT=xT_sb[kt][cc][:, co * 128:(co + 1) * 128],
                        rhs=w_sb["v"][kt][:],
                        start=(kt == 0), stop=(kt == KD - 1),
                    )
                nc.vector.tensor_add(
                    v_sb[:, st, :, 0:A],
                    psv[:].rearrange("p (h a) -> p h a", a=A),
                    bvb_sb[:].rearrange("p (h a) -> p h a", a=A),
                )

            def scores_group(p, qc, ng):
                # S^T for heads 2p (rows 0:64) / 2p+1 (rows 64:128); the two
                # K=64 matmuls hit disjoint PE row groups and co-run.
                # exp issued immediately after each head's scores.
                qs = slice(qc * QW, (qc + 1) * QW)
                sts = [stp.tile([128, KT_PER_ST * QW], f32,
                                name=f"st{hh}", tag=f"st{hh}")
                       for hh in range(2)]
                # full-K contraction against zero-padded qT keeps the PE in
                # uniform 128x128 mode (zeros annihilate the other head)
                for jj in range(KT_PER_ST):
                    kt = ng * KT_PER_ST + jj
                    for hh in range(2):
                        nc.tensor.matmul(
                            sts[hh][:, jj * QW:(jj + 1) * QW],
                            lhsT=kT_sb[p][:, kt * 128:(kt + 1) * 128],
                            rhs=qTz_sb[p][hh][:, qs],
                            start=True, stop=True,
                        )
                pts = []
                for hh in range(2):
                    pt = ptpool.tile([128, KT_PER_ST * QW], bf16,
                                     name=f"pt{hh}", tag=f"pt{hh}")
                    nc.scalar.activation(pt[:], sts[hh][:], AF.Exp,
                                         scale=0.125)
                    pts.append(pt)
                return pts

            def av_group(p, ng, avs, pts):
                heads = (2 * p, 2 * p + 1)
                for jj in range(KT_PER_ST):
                    kt = ng * KT_PER_ST + jj
                    for hh in range(2):
                        nc.tensor.matmul(
                            avs[hh][:],
                            lhsT=v_sb[:, kt, heads[hh], :],
                            rhs=pts[hh][:, jj * QW:(jj + 1) * QW],
                            start=(kt == 0), stop=(kt == NS - 1),
                        )

            def norm_unit(p, qc, avs):
                # attn[a, q] = av[a, q] * (1 / av[A, q]); recip on DVE,
                # column-broadcast on GpSimd, one DVE multiply.
                qs = slice(qc * QW, (qc + 1) * QW)
                for hh in range(2):
                    av = avs[hh]
                    off = hh * A
                    # custom-DVE recip must read SBUF (PSUM source gave
                    # garbage on HW) — copy the denominator row out first
                    den = spool.tile([1, QW], f32, name="den", tag="den")
                    nc.vector.tensor_copy(den[:], av[A:A + 1, :])
                    rec = spool.tile([1, QW], f32, name="rec", tag="rec")
                    nc.vector.reciprocal_approx_fast(rec[:], den[:])
                    bcd = spool.tile([A, QW], f32, name="bcd", tag="bcd")
                    nc.gpsimd.partition_broadcast(bcd[:], rec[:], channels=A)
                    nc.vector.tensor_mul(
                        attn_sb[p][off:off + A, qs], av[0:A, :], bcd[:])

            def fc_group(mt):
                # out rows [mt*128, (mt+1)*128) = attT^T @ Wo + bo
                ob = opool.tile([128, d], f32, name="ob", tag="ob")
                for nn in range(d // OW):
                    ns_ = slice(nn * OW, (nn + 1) * OW)
                    ps = pp1.tile([128, OW], f32, name="ps_o", tag="ps_qk")
                    for kt in range(MC):
                        nc.tensor.matmul(
                            ps[:],
                            lhsT=attn_sb[kt][:, mt * 128:(mt + 1) * 128],
                            rhs=wo_sb[kt][:, ns_],
                            start=(kt == 0), stop=(kt == MC - 1),
                        )
                    nc.vector.tensor_add(ob[:, ns_], ps[:], bob_sb[:, ns_])
                nc.sync.dma_start(out_d[mt * 128:(mt + 1) * 128, :], ob[:])

            # ---------------- pipelined schedule ----------------
            # filler: PE work drained into slack inside ACT-bound stretches
            filler = deque()
            done = set()

            def push(key, fn):
                filler.append((key, fn))

            def drain(n=1):
                for _ in range(n):
                    if not filler:
                        return
                    key, fn = filler.popleft()
                    fn()
                    done.add(key)

            def drain_until(key):
                while key not in done and filler:
                    k, fn = filler.popleft()
                    fn()
                    done.add(k)

            for st in range(16):
                push(("v", st), (lambda st=st: proj_v_group(st)))
            for qc in range(1, QC):
                push(("qT", 0, qc), (lambda qc=qc: proj_qk_group("q", 0, qc)))
            for qc in range(QC):
                push(("kT", 1, qc), (lambda qc=qc: proj_qk_group("k", 1, qc)))
            for qc in range(QC):
                push(("qT", 1, qc), (lambda qc=qc: proj_qk_group("q", 1, qc)))

            units = [(0, 0), (0, 1), (0, 2), (1, 0), (0, 3), (1, 1),
                     (1, 2), (1, 3)]
            # fc chunk qc becomes ready once (1, qc) is normalized
            fc_ready_after = {(1, qc): qc for qc in range(QC)}

            # prologue: kT(0) chunk 0, qT(0) chunk 0
            proj_qk_group("k", 0, 0)
            done.add(("kT", 0, 0))
            proj_qk_group("q", 0, 0)
            done.add(("qT", 0, 0))

            for i, (p, qc) in enumerate(units):
                # prereqs of this unit's scores
                if p == 1:
                    drain_until(("kT", 1, QC - 1))
                    drain_until(("qT", 1, qc))
                else:
                    drain_until(("qT", 0, qc))
                avs = [avp.tile([128, QW], f32, name=f"av{hh}",
                                tag=f"av{hh}") for hh in range(2)]
                pts_list = []
                # AV lags its own scores by one ng: av(ng-1) runs right as
                # exp(ng-1) completes, so ACT never waits across phases
                for ng in range(NG):
                    need_kt = ("kT", p, min(QC - 1, (ng * KT_PER_ST + 1) // 4))
                    if p == 0 and need_kt not in done and i == 0:
                        proj_qk_group("k", 0, need_kt[2])
                        done.add(need_kt)
                    pts_list.append(scores_group(p, qc, ng))
                    if i == 0:
                        drain_until(("v", 2 * ng + 1))
                    else:
                        drain(1)
                    if ng > 0:
                        av_group(p, ng - 1, avs, pts_list[ng - 1])
                av_group(p, NG - 1, avs, pts_list[NG - 1])
                norm_unit(p, qc, avs)
                if (p, qc) in fc_ready_after:
                    fqc = fc_ready_after[(p, qc)]
                    for mt in range(4 * fqc, 4 * fqc + 4):
                        filler.appendleft(
                            (("fc", mt), (lambda mt=mt: fc_group(mt))))

            # tail: leftover filler (last fc chunk)
            while filler:
                drain(1)

    nc.compile()
    return nc


def make_in_maps(x, Wq, bq, Wk, bk, Wv, bv, Wo, bo, n_cores=N_CORES):
    import ml_dtypes
    cf = ml_dtypes.bfloat16
    d = x.shape[2]
    MC = C // 128
    f = np.float32
    in_maps = []
    for core in range(n_cores):
        b, g = divmod(core, GROUPS)
        cs = slice(g * C, (g + 1) * C)
        bob = np.broadcast_to(bo, (128, d)).astype(f) if g == 0 else \
            np.zeros((128, d), f)
        m = {
            "xT": np.ascontiguousarray(x[b].T.astype(cf)),
            "wq": np.ascontiguousarray(Wq[:, cs].astype(cf)),
            "wk": np.ascontiguousarray(Wk[:, cs].astype(cf)),
            "wv": np.ascontiguousarray(Wv[:, cs].astype(cf)),
            "wo": np.ascontiguousarray(Wo[cs].astype(cf)),
            "bqs": np.ascontiguousarray(bq[cs].reshape(MC, 128).T, dtype=f),
            "bks": np.ascontiguousarray(bk[cs].reshape(MC, 128).T, dtype=f),
            "bvb": np.ascontiguousarray(np.broadcast_to(bv[cs], (128, C)), dtype=f),
            "bob": np.ascontiguousarray(bob),
        }
        in_maps.append(m)
    return in_maps


_nc_cache = {}


def _get_nc():
    if "nc" not in _nc_cache:
        _nc_cache["nc"] = build_nc()
    return _nc_cache["nc"]


def assemble(results):
    out = np.empty((B, S, D), np.float32)
    for b in range(B):
        acc = results[b * GROUPS]["out"].copy()
        for g in range(1, GROUPS):
            acc += results[b * GROUPS + g]["out"]
        out[b] = acc
    return out


def kernel(x, Wq, bq, Wk, bk, Wv, bv, Wo, bo, _trace=False, _mode=None):
    from concourse.bass_utils import run_bass_kernel_spmd

    nc = _get_nc()
    in_maps = make_in_maps(x, Wq, bq, Wk, bk, Wv, bv, Wo, bo)
    res = run_bass_kernel_spmd(nc, in_maps, core_ids=list(range(N_CORES)),
                               trace=_trace)
    _nc_cache["last_result"] = res
    return assemble(res.results)
